# revision 83
# baseline (speedup 1.0000x reference)
"""Trainium2 Bass kernel for MinimalHGRNCore (BitLinear projections + HGRN scan).

Contract: kernel(**inputs) takes FULL unsharded numpy inputs and returns the
FULL (B, L, H) float32 output.

Sharding: 8 cores = (batch b in 0..3) x (E-half eh in 0..1).
Each core processes all L tokens of one batch and half of the E features for
the i/f/g projections + recurrence; the final Wo projection is split by
output-H half, contracting over full E.  The remote-half quantized y
activations arrive via a pair-wise AllGather; the final matmul runs in two
passes (local half from SBUF while the collective flies, then the remote
half) so the PE never waits on the wire.

Exactness: act_quant produces integers in [-127,127] and weight_quant values
in {-1,0,+1} * scales.  Both are exactly representable in fp16, so the PE
matmuls run in fp16 with fp32 PSUM accumulation == exact integer arithmetic
(|sum| <= 2048*127 < 2^24).  Rounding uses the fp32 magic-number trick
(x + 1.5*2^23 rounds to nearest-even integer), matching jnp.round.

Layout notes (host side, in make_in_maps):
- Wi/Wf/Wg halves ship as block layout [128, JE, KH*128] so each P3 j-strip
  DMA is 8 KB/partition contiguous.
- Wo half ships as [E, HL] with the core's LOCAL E-half rows first, so the
  two-pass final matmul addresses its weight strips core-independently.
"""

from contextlib import ExitStack
from dataclasses import dataclass

import numpy as np

import concourse.bass as bass
import concourse.mybir as mybir
import concourse.tile as tile
from concourse import bacc
from concourse.masks import make_identity

F32 = mybir.dt.float32
F16 = mybir.dt.float16
AF = mybir.ActivationFunctionType
ALU = mybir.AluOpType
AX = mybir.AxisListType

M32 = 12582912.0  # 1.5 * 2**23: fp32 add rounds to nearest-even integer exactly


@dataclass
class Cfg:
    T: int = 2048      # tokens per core (= L of its batch)
    H: int = 2048      # input hidden dim (contraction for i/f/g)
    EL: int = 1024     # local E features per core (= E/2)
    n_cores: int = 8
    silu_lut: bool = True
    ones_rms: bool = False   # rms_w == 1 -> skip the s*rms multiply
    ones_no: bool = False    # norm_o == 1 -> skip |u|*norm_o^2 scaling

    @property
    def E(self):
        return 2 * self.EL

    @property
    def HL(self):
        return self.H // 2

    @property
    def MT(self):
        return self.T // 128

    @property
    def KH(self):
        return self.H // 128

    @property
    def JE(self):
        return self.EL // 128

    @property
    def KE(self):
        return self.E // 128

    @property
    def NT(self):
        return min(512, self.T)

    @property
    def NN(self):
        return self.T // self.NT

    @property
    def pairs(self):
        return [[2 * i, 2 * i + 1] for i in range(self.n_cores // 2)]


def build_hgrn(tc: tile.TileContext, outs: dict, ins: dict, cfg: Cfg):
    """Emit the SPMD program (identical on every core) into TileContext tc."""
    nc = tc.nc
    c = cfg
    x = ins["x"]
    wb = {"f": ins["wbf"], "i": ins["wbi"], "g": ins["wbg"]}
    woT = ins["woT"]
    rms_w_h, norm_o_h = ins["rms_w_h"], ins["norm_o_h"]
    out = outs["out"]

    ctx = ExitStack()
    with ctx:
        const = ctx.enter_context(tc.tile_pool(name="const", bufs=1))
        small = ctx.enter_context(tc.tile_pool(name="small", bufs=2))
        dram = ctx.enter_context(tc.tile_pool(name="dram", bufs=1, space="DRAM"))

        ones_row = const.tile([1, 128], F32, tag="ones_row")
        nc.vector.memset(ones_row[:], 1.0)
        ones_col = const.tile([128, 1], F32, tag="ones_col")
        nc.vector.memset(ones_col[:], 1.0)
        ident16 = const.tile([128, 128], F16, tag="ident16")
        make_identity(nc, ident16[:])
        ident32 = const.tile([128, 128], F32, tag="ident32")
        make_identity(nc, ident32[:])

        norm_o_row = const.tile([1, c.EL], F32, tag="norm_o_row")
        nc.sync.dma_start(norm_o_row[0:1, :],
                          norm_o_h[:].rearrange("(a t) -> a t", a=1))
        rms_cols = const.tile([128, c.JE], F32, tag="rms_cols")
        norm_o_cols = const.tile([128, c.JE], F32, tag="norm_o_cols")
        nc.sync.dma_start(rms_cols[:],
                          rms_w_h[:].rearrange("(j p) -> p j", p=128))
        nc.sync.dma_start(norm_o_cols[:],
                          norm_o_h[:].rearrange("(j p) -> p j", p=128))
        no2_cols = const.tile([128, c.JE], F32, tag="no2_cols")
        nc.vector.tensor_tensor(no2_cols[:], norm_o_cols[:], norm_o_cols[:],
                                ALU.mult)

        # ------------------------------------------------------------------
        # DRAM bounce tensors for the collectives + spills
        # ------------------------------------------------------------------
        cc1_in = {k: dram.tile([1, 1], F32, tag=f"cc1i_{k}", name=f"cc1i_{k}")
                  for k in ("f", "i", "g", "o")}
        cc1_out = {k: dram.tile([2, 1], F32, tag=f"cc1o_{k}", name=f"cc1o_{k}")
                   for k in ("f", "i", "g", "o")}
        u_dram = dram.tile([c.EL, c.T], F32, tag="u_dram")
        cc2x_in = dram.tile([2, c.T], F32, tag="cc2x_in")
        cc2x_out = dram.tile([2, 2, c.T], F32, tag="cc2x_out")
        cc2y_in = dram.tile([1, c.T], F32, tag="cc2y_in")
        cc2y_out = dram.tile([2, 1, c.T], F32, tag="cc2y_out")
        I8 = mybir.dt.int8
        NC3 = 2  # number of cc3 chunks
        J_CH = c.JE // NC3  # strips per chunk
        cc3_in = [dram.tile([J_CH * 128, c.T], I8, tag=f"cc3i{h}",
                            name=f"cc3i{h}") for h in range(NC3)]
        cc3_out = [dram.tile([2, J_CH * 128, c.T], I8, tag=f"cc3o{h}",
                             name=f"cc3o{h}") for h in range(NC3)]
        xq_dram = dram.tile([c.T, c.H], F16, tag="xq_dram")
        acc_dram = dram.tile([c.T, c.HL], F32, tag="acc_dram")
        dscr = dram.tile([c.T], F32, tag="dscr")
        cscr = dram.tile([c.T], F32, tag="cscr")

        wsums = const.tile([1, 4], F32, tag="wsums")  # f, i, g, o totals
        d_all = const.tile([128, c.MT], F32, tag="d_all")  # 1/scale_tok cols

        ssq_s_cols = const.tile([128, c.MT], F32, tag="ssq_s_cols")
        ssq_u_cols = const.tile([128, c.MT], F32, tag="ssq_u_cols")
        vmax_cols = const.tile([128, c.MT], F32, tag="vmax_cols")

        wi_idx = {"f": 0, "i": 1, "g": 2, "o": 3}
        wcols = const.tile([128, 4 * 8], F32, tag="wcols")  # abs-sum strips

        def wsum_finish(key):
            """Reduce this weight's 8 abs-sum cols to wsums[0, idx], then
            AllGather with the pair partner."""
            idx = wi_idx[key]
            tot = small.tile([128, 1], F32, tag="wfin_tot", name=f"wt_{key}")
            nc.vector.tensor_reduce(
                tot[:], wcols[:, idx * 8:(idx + 1) * 8], AX.X, ALU.add)
            with tc.tile_pool(name="wf_ps", bufs=1, space="PSUM") as pp:
                ps = pp.tile([1, 1], F32, tag="wf_ps", name=f"wfp_{key}")
                nc.tensor.matmul(ps[:], ones_col[:], tot[:], start=True,
                                 stop=True)
                nc.scalar.copy(wsums[0:1, idx:idx + 1], ps[:])
            nc.sync.dma_start(cc1_in[key][:], wsums[0:1, idx:idx + 1])
            nc.gpsimd.collective_compute(
                "AllGather", ALU.bypass, replica_groups=c.pairs,
                ins=[cc1_in[key].opt()], outs=[cc1_out[key].opt()])

        def bcast_col(src_ap, tag):
            t = const.tile([128, 1], F32, tag=tag)
            with tc.tile_pool(name="bc_ps", bufs=1, space="PSUM") as pp:
                ps = pp.tile([128, 1], F32, tag="bc_ps")
                nc.tensor.matmul(ps[:], ones_row[:], src_ap, start=True,
                                 stop=True)
                nc.scalar.copy(t[:], ps[:])
            return t

        n_w_elems = float(c.H) * float(c.E)

        def finish_scale(key):
            """cc1_out[key] [2,1] -> (s_col, m_col) [128,1] broadcasts."""
            a = small.tile([1, 2], F32, tag="fs_a", name=f"fs_a_{key}")
            nc.sync.dma_start(a[0:1, 0:1], cc1_out[key][0:1, :])
            nc.sync.dma_start(a[0:1, 1:2], cc1_out[key][1:2, :])
            m = const.tile([1, 1], F32, tag=f"fs_m_{key}")
            nc.vector.tensor_reduce(m[:], a[:], AX.X, ALU.add)
            nc.vector.tensor_scalar(m[:], m[:], 1.0 / n_w_elems, 1e-5,
                                    ALU.mult, ALU.max)
            s = const.tile([1, 1], F32, tag=f"fs_s_{key}")
            nc.vector.reciprocal(s[:], m[:])
            return (bcast_col(s[0:1, 0:1], f"sc_{key}"),
                    bcast_col(m[0:1, 0:1], f"mc_{key}"))

        # ===== long-lived big tiles (scoped: closed before P4b/P5) =====
        stats_cm = tc.tile_pool(name="stats", bufs=1)
        stats = stats_cm.__enter__()
        sq_acc_s = stats.tile([128, c.T], F32, tag="sq_acc_s")
        sq_acc_u = stats.tile([128, c.T], F32, tag="sq_acc_u")
        vmax = stats.tile([128, c.T], F32, tag="vmax")

        xqTp_cm = tc.tile_pool(name="xqTp", bufs=1)
        xqTp = xqTp_cm.__enter__()
        xqT_t = xqTp.tile([128, c.KH, c.T], F16, tag="xqT")

        # ------------------------------------------------------------------
        # Preamble: x act_quant (PE-transposed into xqT) interleaved with
        # weight |.| strip sums.  Strip stream order: f(8) i(8) g(8) o(8),
        # 2 strips per m-chunk; scale collectives fire as each weight's
        # strips complete.
        # ------------------------------------------------------------------
        def w_strip_dma(wt, key, s):
            if key == "o":
                # woT [E, HL]: 2 k-rows per strip -> [128, 2*HL]
                for a in range(2):
                    nc.sync.dma_start(
                        wt[:, a * c.HL:(a + 1) * c.HL],
                        woT[s * 256 + a * 128:s * 256 + (a + 1) * 128, :])
            else:
                nc.sync.dma_start(wt[:], wb[key][:, s, :])

        # f/i/g strips first (3 per m-chunk for m<8) so the f scale lands
        # by m=2 and all three by m=7; wo strips ride m=8..15
        strip_stream = [(k, s) for k in ("f", "i", "g", "o") for s in range(8)]

        def strips_for_m(m):
            if m < 8:
                return [si for si in (3 * m, 3 * m + 1, 3 * m + 2)
                        if si < 24]
            return [24 + (m - 8)]

        prio_anchor = [None]
        with tc.tile_pool(name="xp", bufs=5) as xp, \
             tc.tile_pool(name="xsq", bufs=2) as xsqp, \
             tc.tile_pool(name="xq16", bufs=2) as xqp, \
             tc.tile_pool(name="wstr", bufs=2) as wstr, \
             tc.tile_pool(name="pcol", bufs=2) as pcol, \
             tc.tile_pool(name="tp_ps", bufs=4, space="PSUM") as tpp:

            for ng in range(c.MT // 4):
                if ng == 2:
                    # P3's emission is priority-anchored here so the
                    # scheduler can interleave its early j's with the
                    # remaining preamble groups
                    prio_anchor[0] = tc.cur_priority
                xts = []
                ssq4 = pcol.tile([128, 4], F32, tag="ssq4", name="ssq4")
                amax4 = pcol.tile([128, 4], F32, tag="amax4", name="amax4")
                for mi in range(4):
                    m = 4 * ng + mi
                    xt = xp.tile([128, c.H], F32, tag="x_t", name="x_t")
                    nc.sync.dma_start(xt[:], x[m * 128:(m + 1) * 128, :])
                    xts.append(xt)
                    sq = xsqp.tile([128, c.H], F32, tag="x_sq", name="x_sq")
                    nc.scalar.activation(sq[:], xt[:], AF.Square,
                                         accum_out=ssq4[:, mi:mi + 1])
                    nc.vector.tensor_reduce(amax4[:, mi:mi + 1], xt[:], AX.X,
                                            ALU.max, apply_absolute_value=True)
                    # weight abs strips (3/m for m<8, then 1/m)
                    for si in strips_for_m(m):
                        wkey, s = strip_stream[si]
                        wt = wstr.tile([128, 2048], F32, tag="wabs_t",
                                       name="wabs_t")
                        w_strip_dma(wt, wkey, s)
                        # |w| in place on ACT, per-partition sum via accum
                        nc.scalar.activation(
                            wt[:], wt[:], AF.Abs,
                            accum_out=wcols[:, wi_idx[wkey] * 8 + s:
                                            wi_idx[wkey] * 8 + s + 1])
                        if s == 7:
                            wsum_finish(wkey)

                # per-group stat chain on [128, 4]
                v = pcol.tile([128, 4], F32, tag="x_v", name="x_v")
                nc.vector.tensor_scalar(v[:], ssq4[:], 1.0 / c.H, 1e-8,
                                        ALU.mult, ALU.add)
                rv = pcol.tile([128, 4], F32, tag="x_rv", name="x_rv")
                nc.vector.reciprocal(rv[:], v[:])
                r0 = pcol.tile([128, 4], F32, tag="x_r0", name="x_r0")
                nc.scalar.sqrt(r0[:], rv[:])
                nt = pcol.tile([128, 4], F32, tag="x_nt", name="x_nt")
                nc.vector.tensor_tensor(nt[:], r0[:], r0[:], ALU.mult)
                nc.vector.tensor_tensor(nt[:], nt[:], v[:], ALU.mult)
                nc.vector.tensor_scalar(nt[:], nt[:], -0.5, 1.5,
                                        ALU.mult, ALU.add)
                rstd = pcol.tile([128, 4], F32, tag="x_rstd", name="x_rstd")
                nc.vector.tensor_tensor(rstd[:], r0[:], nt[:], ALU.mult)
                amx = pcol.tile([128, 4], F32, tag="x_amx", name="x_amx")
                nc.vector.tensor_tensor(amx[:], amax4[:], rstd[:], ALU.mult)
                nc.vector.tensor_scalar(amx[:], amx[:], 1e-5, None, ALU.max)
                ra = pcol.tile([128, 4], F32, tag="x_ra", name="x_ra")
                nc.vector.reciprocal(ra[:], amx[:])
                sc = pcol.tile([128, 4], F32, tag="x_sc", name="x_sc")
                nc.vector.tensor_scalar(sc[:], ra[:], 127.0, None, ALU.mult)
                cc4 = pcol.tile([128, 4], F32, tag="x_cc", name="x_cc")
                nc.vector.tensor_tensor(cc4[:], sc[:], rstd[:], ALU.mult)
                nc.vector.reciprocal(d_all[:, 4 * ng:4 * ng + 4], sc[:])

                for mi in range(4):
                    m = 4 * ng + mi
                    xt = xts[mi]
                    xqf = xsqp.tile([128, c.H], F32, tag="x_sq", name="xqf")
                    nc.vector.tensor_scalar(xqf[:], xt[:], cc4[:, mi:mi + 1],
                                            M32, ALU.mult, ALU.add)
                    # fold the per-token dequant 1/scale into the quantized
                    # activations: xq = (round(x*cc)) * d_tok, fp16
                    xq = xqp.tile([128, c.H], F16, tag="xq16", name="xq16")
                    nc.vector.tensor_scalar(xq[:], xqf[:], M32, None,
                                            ALU.subtract)
                    nc.vector.tensor_scalar(xq[:], xq[:], d_all[:, m:m + 1],
                                            None, ALU.mult)
                    if "d1_xq" in outs:
                        nc.sync.dma_start(
                            outs["d1_xq"][m * 128:(m + 1) * 128, :], xq[:])
                    # PE transpose into xqT, 4 k-blocks per PSUM bank
                    for kq in range(c.KH // 4):
                        ps = tpp.tile([128, 512], F16, tag="tp_ps",
                                      name="tp")
                        for i4 in range(4):
                            k = 4 * kq + i4
                            nc.tensor.transpose(
                                ps[:, i4 * 128:(i4 + 1) * 128],
                                xq[:, k * 128:(k + 1) * 128], ident16[:])
                        nc.vector.tensor_copy(
                            xqT_t[:, 4 * kq:4 * kq + 4,
                                  m * 128:(m + 1) * 128],
                            ps[:].rearrange("p (a b) -> p a b", b=128))

        s_wf_c, m_wf_c = finish_scale("f")
        s_wi_c, m_wi_c = finish_scale("i")
        s_wg_c, m_wg_c = finish_scale("g")
        s_wo_c, m_wo_c = finish_scale("o")

        # ------------------------------------------------------------------
        # P3: per local-e chunk: W strips, i/f/g matmuls, gates, scan, u,
        #     stat accumulation.  u spilled to DRAM.
        # ------------------------------------------------------------------
        p3_prio = ExitStack()
        if prio_anchor[0] is not None and tc.cur_priority > prio_anchor[0]:
            p3_prio.enter_context(
                tc.high_priority(offset=tc.cur_priority - prio_anchor[0]))
        with p3_prio, \
             tc.tile_pool(name="p3", bufs=2) as p3, \
             tc.tile_pool(name="p3w", bufs=2) as p3w, \
             tc.tile_pool(name="p3q", bufs=4) as p3q, \
             tc.tile_pool(name="p3s", bufs=3) as p3s, \
             tc.tile_pool(name="p3a", bufs=6) as p3a, \
             tc.tile_pool(name="mm_ps", bufs=8, space="PSUM") as mm_ps:

            def quant_w_strip(key, s_col, j, nm):
                strip = p3w.tile([128, c.KH * 128], F32, tag="w_strip",
                                 name="w_strip")
                nc.sync.dma_start(strip[:], wb[key][:, j, :])
                nc.scalar.activation(strip[:], strip[:], AF.Copy, bias=M32,
                                     scale=s_col[:])
                nc.vector.tensor_scalar(strip[:], strip[:], M32, 1.0,
                                        ALU.subtract, ALU.min)
                q = p3q.tile([128, c.KH, 128], F16, tag="wq_strip", name=nm)
                nc.vector.tensor_scalar(q[:].rearrange("p k e -> p (k e)"),
                                        strip[:], -1.0, None, ALU.max)
                return q

            def proj_all(wq):
                # k-outer so each LDWEIGHTS covers NN matmuls
                pss = [mm_ps.tile([128, c.NT], F32, tag="proj_ps",
                                  name=f"proj_ps{n}") for n in range(c.NN)]
                for k in range(c.KH):
                    for n in range(c.NN):
                        nc.tensor.matmul(
                            pss[n][:], wq[:, k, :],
                            xqT_t[:, k, n * c.NT:(n + 1) * c.NT],
                            start=(k == 0), stop=(k == c.KH - 1),
                            skip_group_check=True)
                return pss

            for j in range(c.JE):
                wq_f = quant_w_strip("f", s_wf_c, j, "wq_f")
                wq_i = quant_w_strip("i", s_wi_c, j, "wq_i")
                wq_g = quant_w_strip("g", s_wg_c, j, "wq_g")

                f_j = p3.tile([128, c.T], F32, tag="bigA", name="f_j")
                ii_j = p3.tile([128, c.T], F32, tag="bigB", name="ii_j")
                pss_f = proj_all(wq_f)
                fms = []
                for n in range(c.NN):  # sigmoid batch (direct from PSUM)
                    sl = bass.ts(n, c.NT)
                    nc.scalar.activation(f_j[:, sl], pss_f[n][:], AF.Sigmoid,
                                         scale=m_wf_c[:])
                    fm = p3a.tile([128, c.NT], F32, tag="act_o", name="fm")
                    nc.vector.tensor_scalar(fm[:], f_j[:, sl], -1.0, 1.0,
                                            ALU.mult, ALU.add)
                    fms.append(fm)
                pss_i = proj_all(wq_i)
                for n in range(c.NN):  # silu batch + ii
                    sl = bass.ts(n, c.NT)
                    si = p3a.tile([128, c.NT], F32, tag="act_o", name="si")
                    nc.scalar.activation(si[:], pss_i[n][:], AF.Silu,
                                         scale=m_wi_c[:])
                    nc.vector.tensor_tensor(ii_j[:, sl], si[:], fms[n][:],
                                            ALU.mult)

                s_j = p3.tile([128, c.T], F32, tag="bigB", name="s_j")
                nc.vector.tensor_tensor_scan(s_j[:], f_j[:], ii_j[:],
                                             0.0, ALU.mult, ALU.add)
                if "d2_f" in outs:
                    nc.sync.dma_start(
                        outs["d2_f"][j * 128:(j + 1) * 128, :], f_j[:])
                if "d3_s" in outs:
                    nc.sync.dma_start(
                        outs["d3_s"][j * 128:(j + 1) * 128, :], s_j[:])

                u_j = p3.tile([128, c.T], F32, tag="bigA", name="u_j")
                pss_g = proj_all(wq_g)
                for n in range(c.NN):  # silu batch (g) + u
                    sl = bass.ts(n, c.NT)
                    gg = p3a.tile([128, c.NT], F32, tag="act_o", name="gg")
                    nc.scalar.activation(gg[:], pss_g[n][:], AF.Silu,
                                         scale=m_wg_c[:])
                    nc.vector.tensor_tensor(u_j[:, sl], gg[:], s_j[:, sl],
                                            ALU.mult)
                    if not c.ones_rms:
                        nc.vector.tensor_scalar(u_j[:, sl], u_j[:, sl],
                                                rms_cols[:, j:j + 1],
                                                None, ALU.mult)
                for n in range(c.NN):  # square batch + stat acc
                    sl = bass.ts(n, c.NT)
                    sq1 = p3s.tile([128, c.NT], F32, tag="sq_scr", name="sq1")
                    nc.scalar.activation(sq1[:], s_j[:, sl], AF.Square)
                    if j == 0:
                        nc.vector.tensor_copy(sq_acc_s[:, sl], sq1[:])
                    else:
                        nc.vector.tensor_tensor(sq_acc_s[:, sl],
                                                sq_acc_s[:, sl], sq1[:],
                                                ALU.add)
                    sq2 = p3s.tile([128, c.NT], F32, tag="sq_scr", name="sq2")
                    nc.scalar.activation(sq2[:], u_j[:, sl], AF.Square)
                    if j == 0:
                        nc.vector.tensor_copy(sq_acc_u[:, sl], sq2[:])
                        if c.ones_no:
                            nc.vector.tensor_copy(vmax[:, sl], sq2[:])
                        else:
                            nc.vector.tensor_scalar(
                                vmax[:, sl], sq2[:], no2_cols[:, 0:1],
                                None, ALU.mult)
                    else:
                        nc.vector.tensor_tensor(sq_acc_u[:, sl],
                                                sq_acc_u[:, sl], sq2[:],
                                                ALU.add)
                        if c.ones_no:
                            nc.vector.tensor_tensor(vmax[:, sl], vmax[:, sl],
                                                    sq2[:], ALU.max)
                        else:
                            va2 = p3s.tile([128, c.NT], F32, tag="sq_scr",
                                           name="va2")
                            nc.vector.tensor_scalar(
                                va2[:], sq2[:], no2_cols[:, j:j + 1],
                                None, ALU.mult)
                            nc.vector.tensor_tensor(vmax[:, sl], vmax[:, sl],
                                                    va2[:], ALU.max)
                nc.sync.dma_start(u_dram[j * 128:(j + 1) * 128, :], u_j[:])
                if "d4_u" in outs:
                    nc.sync.dma_start(
                        outs["d4_u"][j * 128:(j + 1) * 128, :], u_j[:])
            p3_end_prio = tc.cur_priority
        tc.cur_priority = max(tc.cur_priority, p3_end_prio)

        xqTp_cm.__exit__(None, None, None)  # free xqT's 8 MB

        # --------------------------------------------------------------
        # P4a: partition-reduce stats -> per-token columns
        # (before woq so the stats pool can close in stack order)
        # --------------------------------------------------------------
        with tc.tile_pool(name="st_ps", bufs=4, space="PSUM") as stp:
            for src_t, dst, op in ((sq_acc_s, ssq_s_cols, ALU.add),
                                   (sq_acc_u, ssq_u_cols, ALU.add),
                                   (vmax, vmax_cols, ALU.max)):
                for mq in range(c.MT // 4):
                    tp = stp.tile([128, 512], F32, tag="st_ps", name="tp")
                    for i4 in range(4):
                        m = 4 * mq + i4
                        nc.tensor.transpose(
                            tp[:, i4 * 128:(i4 + 1) * 128],
                            src_t[:, m * 128:(m + 1) * 128], ident32[:])
                    nc.vector.tensor_reduce(
                        dst[:, 4 * mq:4 * mq + 4],
                        tp[:].rearrange("p (a b) -> p a b", b=128),
                        AX.X, op)
        stats_cm.__exit__(None, None, None)  # free the 4 MB of stat tiles

        # --------------------------------------------------------------
        # woq quant: Wo full half -> ternary fp16 [128, KE, HL]
        # --------------------------------------------------------------
        woqp = ctx.enter_context(tc.tile_pool(name="woqp", bufs=1))
        woq = woqp.tile([128, c.KE, c.HL], F16, tag="woq")
        with tc.tile_pool(name="wo_ld", bufs=2) as wol:
            for k in range(c.KE):
                wt = wol.tile([128, c.HL], F32, tag="wo_t", name="wo_t")
                nc.sync.dma_start(wt[:], woT[k * 128:(k + 1) * 128, :])
                nc.scalar.activation(wt[:], wt[:], AF.Copy, bias=M32,
                                     scale=s_wo_c[:])
                nc.vector.tensor_scalar(wt[:], wt[:], M32, 1.0,
                                        ALU.subtract, ALU.min)
                nc.vector.tensor_scalar(woq[:, k, :], wt[:], -1.0, None,
                                        ALU.max)

        # ssq exchange fires first; the amax sqrt-chain and its (smaller)
        # exchange overlap it, and the rstd chain overlaps that
        for row, cols in ((0, ssq_s_cols), (1, ssq_u_cols)):
            nc.sync.dma_start(
                cc2x_in[row, :].rearrange("(m p) -> p m", p=128), cols[:])
        nc.gpsimd.collective_compute(
            "AllGather", ALU.bypass, replica_groups=c.pairs,
            ins=[cc2x_in.opt()], outs=[cc2x_out.opt()])

        amax_cols = const.tile([128, c.MT], F32, tag="amax_cols")
        a0 = const.tile([128, c.MT], F32, tag="amax_a0")
        nc.scalar.sqrt(a0[:], vmax_cols[:])
        ar = const.tile([128, c.MT], F32, tag="amax_ar")
        nc.vector.tensor_scalar(ar[:], a0[:], 1e-30, None, ALU.max)
        nc.vector.reciprocal(ar[:], ar[:])
        nc.vector.tensor_tensor(ar[:], ar[:], vmax_cols[:], ALU.mult)
        nc.vector.tensor_tensor(ar[:], ar[:], a0[:], ALU.add)
        nc.vector.tensor_scalar(amax_cols[:], ar[:], 0.5, None, ALU.mult)
        nc.sync.dma_start(
            cc2y_in[0, :].rearrange("(m p) -> p m", p=128), amax_cols[:])
        nc.gpsimd.collective_compute(
            "AllGather", ALU.bypass, replica_groups=c.pairs,
            ins=[cc2y_in.opt()], outs=[cc2y_out.opt()])

        def load_stat_cols(cc_out, row, op, tag):
            a = small.tile([128, c.MT], F32, tag=tag + "_a", name=tag + "_a")
            b = small.tile([128, c.MT], F32, tag=tag + "_b", name=tag + "_b")
            nc.sync.dma_start(a[:],
                              cc_out[0, row, :].rearrange("(m p) -> p m",
                                                          p=128))
            nc.sync.dma_start(b[:],
                              cc_out[1, row, :].rearrange("(m p) -> p m",
                                                          p=128))
            r = small.tile([128, c.MT], F32, tag=tag, name=tag)
            nc.vector.tensor_tensor(r[:], a[:], b[:], op)
            return r

        def refine_rsqrt_cols(v_ap, r0_ap, out_ap, tag):
            nt = small.tile([128, c.MT], F32, tag=tag)
            nc.vector.tensor_tensor(nt[:], r0_ap, r0_ap, ALU.mult)
            nc.vector.tensor_tensor(nt[:], nt[:], v_ap, ALU.mult)
            nc.vector.tensor_scalar(nt[:], nt[:], -0.5, 1.5, ALU.mult,
                                    ALU.add)
            nc.vector.tensor_tensor(out_ap, r0_ap, nt[:], ALU.mult)

        ssq_s = load_stat_cols(cc2x_out, 0, ALU.add, "ssq_s")
        ssq_u = load_stat_cols(cc2x_out, 1, ALU.add, "ssq_u")
        amax_y = load_stat_cols(cc2y_out, 0, ALU.max, "amax_y")

        ms = small.tile([128, c.MT], F32, tag="ms")
        nc.vector.tensor_scalar(ms[:], ssq_s[:], 1.0 / c.E, 1e-5, ALU.mult,
                                ALU.add)
        rms_i = small.tile([128, c.MT], F32, tag="rms_i")
        nc.vector.reciprocal(rms_i[:], ms[:])
        rstd_s0 = small.tile([128, c.MT], F32, tag="rstd_s0")
        nc.scalar.sqrt(rstd_s0[:], rms_i[:])
        rstd_s = small.tile([128, c.MT], F32, tag="rstd_s")
        refine_rsqrt_cols(ms[:], rstd_s0[:], rstd_s[:], "nt_s")

        m2 = small.tile([128, c.MT], F32, tag="m2")
        nc.vector.tensor_scalar(m2[:], ssq_u[:], 1.0 / c.E, None, ALU.mult)
        r2 = small.tile([128, c.MT], F32, tag="r2")
        nc.vector.tensor_tensor(r2[:], rstd_s[:], rstd_s[:], ALU.mult)
        nc.vector.tensor_tensor(m2[:], m2[:], r2[:], ALU.mult)
        nc.vector.tensor_scalar(m2[:], m2[:], 1e-8, None, ALU.add)
        m2i = small.tile([128, c.MT], F32, tag="m2i")
        nc.vector.reciprocal(m2i[:], m2[:])
        rsty0 = small.tile([128, c.MT], F32, tag="rsty0")
        nc.scalar.sqrt(rsty0[:], m2i[:])
        rsty = small.tile([128, c.MT], F32, tag="rsty")
        refine_rsqrt_cols(m2[:], rsty0[:], rsty[:], "nt_y")

        rr = small.tile([128, c.MT], F32, tag="rr")
        nc.vector.tensor_tensor(rr[:], rstd_s[:], rsty[:], ALU.mult)
        av = small.tile([128, c.MT], F32, tag="av")
        nc.vector.tensor_tensor(av[:], amax_y[:], rr[:], ALU.mult)
        nc.vector.tensor_scalar(av[:], av[:], 1e-5, None, ALU.max)
        avi = small.tile([128, c.MT], F32, tag="avi")
        nc.vector.reciprocal(avi[:], av[:])
        sc_y = small.tile([128, c.MT], F32, tag="sc_y")
        nc.vector.tensor_scalar(sc_y[:], avi[:], 127.0, None, ALU.mult)
        c_y = small.tile([128, c.MT], F32, tag="c_y")
        nc.vector.tensor_tensor(c_y[:], rr[:], sc_y[:], ALU.mult)
        d_y = const.tile([128, c.MT], F32, tag="d_y")
        nc.vector.reciprocal(d_y[:], sc_y[:])
        nc.vector.tensor_scalar(d_y[:], d_y[:], m_wo_c[:], None, ALU.mult)

        if not c.ones_no:
            nc.sync.dma_start(cscr[:].rearrange("(m p) -> p m", p=128),
                              c_y[:])
        ones128 = const.tile([128, 128], F32, tag="ones128")
        nc.vector.memset(ones128[:], 1.0)

        # ------------------------------------------------------------------
        # P4b + P5: quantize y per strip, chunked AllGather, two-pass final
        # matmul (pass A: local strips from SBUF with wave-0 pipelining;
        # pass B: remote strips = row0 + row1 - local).
        # ------------------------------------------------------------------
        with tc.tile_pool(name="yq", bufs=c.JE) as yqp, \
             tc.tile_pool(name="yq8p", bufs=2) as yq8p, \
             tc.tile_pool(name="rem", bufs=c.JE) as remp, \
             tc.tile_pool(name="accp", bufs=1) as accp, \
             tc.tile_pool(name="u_rdp", bufs=4) as urdp, \
             tc.tile_pool(name="ccrd", bufs=2) as ccrd, \
             tc.tile_pool(name="out_sb", bufs=3) as osb, \
             tc.tile_pool(name="nc_ps", bufs=2, space="PSUM") as ncp, \
             tc.tile_pool(name="out_ps", bufs=4, space="PSUM") as ops:

            c_row = None
            if not c.ones_no:
                c_row = const.tile([1, c.T], F32, tag="c_row")
                nc.sync.dma_start(c_row[0:1, :],
                                  cscr[:].rearrange("(a t) -> a t", a=1))

            ncb = None
            if c.ones_no:
                # norm_o == 1: the quant scale c_y broadcast is
                # j-independent; build it by PE-transposing per-partition
                # broadcasts of the c_y columns (no DRAM bounce)
                ncb = accp.tile([128, c.T], F32, tag="ncb")
                for m in range(c.MT):
                    rb = osb.tile([128, 128], F32, tag="ncb_rb", name="rb")
                    nc.vector.tensor_scalar(rb[:], ones128[:],
                                            c_y[:, m:m + 1], None, ALU.mult)
                    ps_nc = ncp.tile([128, 128], F32, tag="nc_ps",
                                     name="nc_ps")
                    nc.tensor.transpose(ps_nc[:], rb[:], ident32[:])
                    nc.scalar.copy(ncb[:, m * 128:(m + 1) * 128], ps_nc[:])

            yqs = []
            yq8s = []
            rems = []
            NQ = 2 if c.ones_no else c.NN  # bigger quant chunks if no PSUM
            QW = c.T // NQ
            for j in range(c.JE):
                yq_j = yqp.tile([128, c.T], F16, tag="yq_j", name=f"yq_{j}")
                yqs.append(yq_j)
                for n in range(NQ):
                    sl = bass.ts(n, QW)
                    u_rd = urdp.tile([128, QW], F32, tag="u_rd",
                                     name="u_rd")
                    nc.sync.dma_start(u_rd[:],
                                      u_dram[j * 128:(j + 1) * 128, sl])
                    q0 = osb.tile([128, QW], F32, tag="q0", name="q0")
                    if c.ones_no:
                        nc.vector.tensor_tensor(q0[:], u_rd[:], ncb[:, sl],
                                                ALU.mult)
                    else:
                        ps_nc = ncp.tile([128, QW], F32, tag="nc_ps",
                                         name="nc_ps")
                        nc.tensor.matmul(
                            ps_nc[:], norm_o_row[0:1, j * 128:(j + 1) * 128],
                            c_row[0:1, sl], start=True, stop=True)
                        nc.vector.tensor_tensor(q0[:], u_rd[:], ps_nc[:],
                                                ALU.mult)
                    nc.vector.tensor_scalar(yq_j[:, sl], q0[:], M32, M32,
                                            ALU.add, ALU.subtract)
                if "d5_yq" in outs:
                    nc.sync.dma_start(
                        outs["d5_yq"][j * 128:(j + 1) * 128, :], yq_j[:])
                # int8 wire format halves the AllGather bytes
                yq8 = yq8p.tile([128, c.T], I8, tag="yq8_j", name="yq8_j")
                nc.vector.tensor_copy(yq8[:], yq_j[:])
                ch = j // J_CH
                jr = j % J_CH
                nc.sync.dma_start(
                    cc3_in[ch][jr * 128:(jr + 1) * 128, :], yq8[:])
                if jr == J_CH - 1:
                    nc.gpsimd.collective_compute(
                        "AllGather", ALU.bypass, replica_groups=c.pairs,
                        ins=[cc3_in[ch].opt()], outs=[cc3_out[ch].opt()])

            # blend remote strips = row0 + row1 - local (exact fp16 ints);
            # emitted after the whole quant loop so the waiting blends don't
            # block the strict-FIFO DVE queue
            for jj in range(c.JE):
                ch, r = jj // J_CH, jj % J_CH
                r0 = ccrd.tile([128, c.T], I8, tag="ccrd", name="r0")
                nc.sync.dma_start(
                    r0[:], cc3_out[ch][0, r * 128:(r + 1) * 128, :])
                r1 = ccrd.tile([128, c.T], I8, tag="ccrd", name="r1")
                nc.sync.dma_start(
                    r1[:], cc3_out[ch][1, r * 128:(r + 1) * 128, :])
                c0 = ccrd.tile([128, c.T], F16, tag="ccup", name="c0")
                nc.vector.tensor_copy(c0[:], r0[:])
                c1 = ccrd.tile([128, c.T], F16, tag="ccup", name="c1")
                nc.vector.tensor_copy(c1[:], r1[:])
                rem = remp.tile([128, c.T], F16, tag="rem_j",
                                name=f"rem_{jj}")
                nc.vector.tensor_tensor(rem[:], c0[:], c1[:], ALU.add)
                nc.vector.tensor_tensor(rem[:], rem[:], yqs[jj][:],
                                        ALU.subtract)
                rems.append(rem)

            # two-pass final matmul: all of pass A (local strips, no wire
            # dependency) before pass B (remote strips via AllGather+blend).
            # n2-inner so each stationary yq/rem chunk serves 2 matmuls.
            NH2 = c.HL // c.NT
            for m in range(c.MT):
                msl = bass.ts(m, 128)
                psA = [ops.tile([128, c.NT], F32, tag="out_ps",
                                name=f"pA{n2}") for n2 in range(NH2)]
                for j in range(c.JE):
                    for n2 in range(NH2):
                        nc.tensor.matmul(
                            psA[n2][:], yqs[j][:, msl],
                            woq[:, j, n2 * c.NT:(n2 + 1) * c.NT],
                            start=(j == 0), stop=(j == c.JE - 1),
                            skip_group_check=True)
                for n2 in range(NH2):
                    nsl = bass.ts(n2, c.NT)
                    aw = osb.tile([128, c.NT], F32, tag="acc_w", name="aw")
                    nc.scalar.copy(aw[:], psA[n2][:])
                    nc.sync.dma_start(acc_dram[msl, nsl], aw[:])
            for m in range(c.MT):
                msl = bass.ts(m, 128)
                # prefetch this m's pass-A partials while its MMs run
                ars = []
                for n2 in range(NH2):
                    ar = osb.tile([128, c.NT], F32, tag="acc_r", name="ar")
                    nc.sync.dma_start(ar[:],
                                      acc_dram[msl, bass.ts(n2, c.NT)])
                    ars.append(ar)
                psB = [ops.tile([128, c.NT], F32, tag="out_ps",
                                name=f"pB{n2}") for n2 in range(NH2)]
                for j in range(c.JE):
                    for n2 in range(NH2):
                        nc.tensor.matmul(
                            psB[n2][:], rems[j][:, msl],
                            woq[:, c.JE + j, n2 * c.NT:(n2 + 1) * c.NT],
                            start=(j == 0), stop=(j == c.JE - 1),
                            skip_group_check=True)
                for n2 in range(NH2):
                    nsl = bass.ts(n2, c.NT)
                    ot = osb.tile([128, c.NT], F32, tag="out_t", name="out_t")
                    nc.vector.tensor_tensor(ot[:], psB[n2][:],
                                            ars[n2][:], ALU.add)
                    ot2 = osb.tile([128, c.NT], F32, tag="out_t2",
                                   name="out_t2")
                    nc.scalar.activation(ot2[:], ot[:], AF.Copy,
                                         scale=d_y[:, m:m + 1])
                    nc.sync.dma_start(out[msl, nsl], ot2[:])


# ----------------------------------------------------------------------
# Host wrapper
# ----------------------------------------------------------------------
_CACHE = {}


def _build_full_program(cfg: Cfg):
    nc = bacc.Bacc(None, target_bir_lowering=False, debug=False,
                   num_devices=cfg.n_cores)
    ins_h = {
        "x": nc.dram_tensor("x", [cfg.T, cfg.H], F32, kind="ExternalInput"),
        "wbi": nc.dram_tensor("wbi", [128, cfg.JE, cfg.KH * 128], F32,
                              kind="ExternalInput"),
        "wbf": nc.dram_tensor("wbf", [128, cfg.JE, cfg.KH * 128], F32,
                              kind="ExternalInput"),
        "wbg": nc.dram_tensor("wbg", [128, cfg.JE, cfg.KH * 128], F32,
                              kind="ExternalInput"),
        "woT": nc.dram_tensor("woT", [cfg.E, cfg.HL], F32,
                              kind="ExternalInput"),
        "rms_w_h": nc.dram_tensor("rms_w_h", [cfg.EL], F32,
                                  kind="ExternalInput"),
        "norm_o_h": nc.dram_tensor("norm_o_h", [cfg.EL], F32,
                                   kind="ExternalInput"),
    }
    out_h = nc.dram_tensor("out", [cfg.T, cfg.HL], F32, kind="ExternalOutput")
    outs = {"out": out_h[:, :]}
    import os
    if os.environ.get("HGRN_DEBUG"):
        for nm, shape, dt in (("d1_xq", [cfg.T, cfg.H], F16),
                              ("d2_f", [cfg.EL, cfg.T], F32),
                              ("d3_s", [cfg.EL, cfg.T], F32),
                              ("d4_u", [cfg.EL, cfg.T], F32),
                              ("d5_yq", [cfg.EL, cfg.T], F16)):
            h = nc.dram_tensor(nm, shape, dt, kind="ExternalOutput")
            outs[nm] = h[:, :]
    with tile.TileContext(nc) as tc:
        build_hgrn(tc, outs,
                   {k: v[tuple(slice(None) for _ in v.shape)]
                    for k, v in ins_h.items()}, cfg)
    nc.compile()
    return nc


def _block_w(w_half_T, cfg):
    """[H, EL] -> [128, JE, KH*128] so strip j is 8KB/partition contiguous."""
    kh, je = cfg.KH, cfg.JE
    return np.ascontiguousarray(
        w_half_T.reshape(kh, 128, je, 128).transpose(1, 2, 0, 3)
        .reshape(128, je, kh * 128))


def make_in_maps(x, Wi, Wf, Wg, Wo, rms_w, norm_o, cfg: Cfg):
    in_maps = []
    for core in range(cfg.n_cores):
        b, eh = core // 2, core % 2
        esl = slice(eh * cfg.EL, (eh + 1) * cfg.EL)
        hsl = slice(eh * cfg.HL, (eh + 1) * cfg.HL)
        woT_full = np.ascontiguousarray(Wo[hsl, :].T)  # [E, HL], global order
        loc = woT_full[eh * cfg.EL:(eh + 1) * cfg.EL]
        rmt = woT_full[(1 - eh) * cfg.EL:(2 - eh) * cfg.EL]
        in_maps.append({
            "x": np.ascontiguousarray(x[b]),
            "wbi": _block_w(np.ascontiguousarray(Wi[esl, :].T), cfg),
            "wbf": _block_w(np.ascontiguousarray(Wf[esl, :].T), cfg),
            "wbg": _block_w(np.ascontiguousarray(Wg[esl, :].T), cfg),
            "woT": np.ascontiguousarray(np.concatenate([loc, rmt], axis=0)),
            "rms_w_h": np.ascontiguousarray(rms_w[esl]),
            "norm_o_h": np.ascontiguousarray(norm_o[esl]),
        })
    return in_maps


def kernel(x, Wi, Wf, Wg, Wo, norm_i, norm_f, norm_g, norm_o, rms_w,
           _trace=False):
    x = np.asarray(x, np.float32)
    for nv in (norm_i, norm_f, norm_g):
        if not np.allclose(np.asarray(nv), 1.0):
            raise NotImplementedError(
                "kernel assumes norm_i == norm_f == norm_g == 1 "
                "(as produced by setup_inputs)")
    B, L, H = x.shape
    cfg = Cfg(T=L, H=H, EL=np.asarray(Wi).shape[0] // 2, n_cores=8,
              ones_rms=bool(np.allclose(np.asarray(rms_w), 1.0)),
              ones_no=bool(np.allclose(np.asarray(norm_o), 1.0)))
    assert B * 2 == cfg.n_cores

    from concourse import bass_utils

    key = (cfg.T, cfg.H, cfg.EL, cfg.ones_rms, cfg.ones_no)
    if key not in _CACHE:
        _CACHE[key] = _build_full_program(cfg)
    nc = _CACHE[key]

    in_maps = make_in_maps(np.asarray(x, np.float32),
                           np.asarray(Wi, np.float32),
                           np.asarray(Wf, np.float32),
                           np.asarray(Wg, np.float32),
                           np.asarray(Wo, np.float32),
                           np.asarray(rms_w, np.float32),
                           np.asarray(norm_o, np.float32), cfg)
    res = bass_utils.run_bass_kernel_spmd(
        nc, in_maps, core_ids=list(range(cfg.n_cores)), trace=_trace)

    out = np.empty((B, L, H), np.float32)
    for core in range(cfg.n_cores):
        b, eh = core // 2, core % 2
        out[b, :, eh * cfg.HL:(eh + 1) * cfg.HL] = res.results[core]["out"]
    kernel.last_raw = res.results
    if _trace:
        kernel.last_exec_time_ns = res.exec_time_ns
        kernel.last_results = res
    return out


# revision 85
# speedup vs baseline: 1.0495x; 1.0495x over previous
"""Trainium2 Bass kernel for MinimalHGRNCore (BitLinear projections + HGRN scan).

Contract: kernel(**inputs) takes FULL unsharded numpy inputs and returns the
FULL (B, L, H) float32 output.

Sharding: 8 cores = (batch b in 0..3) x (E-half eh in 0..1).
Each core processes all L tokens of one batch and half of the E features for
the i/f/g projections + recurrence; the final Wo projection is split by
output-H half, contracting over full E.  The remote-half quantized y
activations arrive via a pair-wise AllGather; the final matmul runs in two
passes (local half from SBUF while the collective flies, then the remote
half) so the PE never waits on the wire.

Exactness: act_quant produces integers in [-127,127] and weight_quant values
in {-1,0,+1} * scales.  Both are exactly representable in fp16, so the PE
matmuls run in fp16 with fp32 PSUM accumulation == exact integer arithmetic
(|sum| <= 2048*127 < 2^24).  Rounding uses the fp32 magic-number trick
(x + 1.5*2^23 rounds to nearest-even integer), matching jnp.round.

Layout notes (host side, in make_in_maps):
- Wi/Wf/Wg halves ship as block layout [128, JE, KH*128] so each P3 j-strip
  DMA is 8 KB/partition contiguous.
- Wo half ships as [E, HL] with the core's LOCAL E-half rows first, so the
  two-pass final matmul addresses its weight strips core-independently.
"""

from contextlib import ExitStack
from dataclasses import dataclass

import numpy as np

import concourse.bass as bass
import concourse.mybir as mybir
import concourse.tile as tile
from concourse import bacc
from concourse.masks import make_identity

F32 = mybir.dt.float32
F16 = mybir.dt.float16
AF = mybir.ActivationFunctionType
ALU = mybir.AluOpType
AX = mybir.AxisListType

M32 = 12582912.0  # 1.5 * 2**23: fp32 add rounds to nearest-even integer exactly


@dataclass
class Cfg:
    T: int = 2048      # tokens per core (= L of its batch)
    H: int = 2048      # input hidden dim (contraction for i/f/g)
    EL: int = 1024     # local E features per core (= E/2)
    n_cores: int = 8
    silu_lut: bool = True
    ones_rms: bool = False   # rms_w == 1 -> skip the s*rms multiply
    ones_no: bool = False    # norm_o == 1 -> skip |u|*norm_o^2 scaling

    @property
    def E(self):
        return 2 * self.EL

    @property
    def HL(self):
        return self.H // 2

    @property
    def MT(self):
        return self.T // 128

    @property
    def KH(self):
        return self.H // 128

    @property
    def JE(self):
        return self.EL // 128

    @property
    def KE(self):
        return self.E // 128

    @property
    def NT(self):
        return min(512, self.T)

    @property
    def NN(self):
        return self.T // self.NT

    @property
    def pairs(self):
        return [[2 * i, 2 * i + 1] for i in range(self.n_cores // 2)]


def build_hgrn(tc: tile.TileContext, outs: dict, ins: dict, cfg: Cfg):
    """Emit the SPMD program (identical on every core) into TileContext tc."""
    nc = tc.nc
    c = cfg
    x = ins["x"]
    wb = {"f": ins["wbf"], "i": ins["wbi"], "g": ins["wbg"]}
    woT = ins["woT"]
    rms_w_h, norm_o_h = ins["rms_w_h"], ins["norm_o_h"]
    out = outs["out"]

    ctx = ExitStack()
    with ctx:
        const = ctx.enter_context(tc.tile_pool(name="const", bufs=1))
        small = ctx.enter_context(tc.tile_pool(name="small", bufs=2))
        dram = ctx.enter_context(tc.tile_pool(name="dram", bufs=1, space="DRAM"))

        ones_row = const.tile([1, 128], F32, tag="ones_row")
        nc.vector.memset(ones_row[:], 1.0)
        ones_col = const.tile([128, 1], F32, tag="ones_col")
        nc.vector.memset(ones_col[:], 1.0)
        ident16 = const.tile([128, 128], F16, tag="ident16")
        make_identity(nc, ident16[:])
        ident32 = const.tile([128, 128], F32, tag="ident32")
        make_identity(nc, ident32[:])

        norm_o_row = const.tile([1, c.EL], F32, tag="norm_o_row")
        nc.sync.dma_start(norm_o_row[0:1, :],
                          norm_o_h[:].rearrange("(a t) -> a t", a=1))
        rms_cols = const.tile([128, c.JE], F32, tag="rms_cols")
        norm_o_cols = const.tile([128, c.JE], F32, tag="norm_o_cols")
        nc.sync.dma_start(rms_cols[:],
                          rms_w_h[:].rearrange("(j p) -> p j", p=128))
        nc.sync.dma_start(norm_o_cols[:],
                          norm_o_h[:].rearrange("(j p) -> p j", p=128))
        no2_cols = const.tile([128, c.JE], F32, tag="no2_cols")
        nc.vector.tensor_tensor(no2_cols[:], norm_o_cols[:], norm_o_cols[:],
                                ALU.mult)

        # ------------------------------------------------------------------
        # DRAM bounce tensors for the collectives + spills
        # ------------------------------------------------------------------
        cc1_in = {k: dram.tile([1, 1], F32, tag=f"cc1i_{k}", name=f"cc1i_{k}")
                  for k in ("f", "i", "g", "o")}
        cc1_out = {k: dram.tile([2, 1], F32, tag=f"cc1o_{k}", name=f"cc1o_{k}")
                   for k in ("f", "i", "g", "o")}
        u_dram = dram.tile([c.EL, c.T], F32, tag="u_dram")
        cc2x_in = dram.tile([2, c.T], F32, tag="cc2x_in")
        cc2x_out = dram.tile([2, 2, c.T], F32, tag="cc2x_out")
        cc2y_in = dram.tile([1, c.T], F32, tag="cc2y_in")
        cc2y_out = dram.tile([2, 1, c.T], F32, tag="cc2y_out")
        I8 = mybir.dt.int8
        NC3 = 2  # number of cc3 chunks
        J_CH = c.JE // NC3  # strips per chunk
        cc3_in = [dram.tile([J_CH * 128, c.T], I8, tag=f"cc3i{h}",
                            name=f"cc3i{h}") for h in range(NC3)]
        cc3_out = [dram.tile([2, J_CH * 128, c.T], I8, tag=f"cc3o{h}",
                             name=f"cc3o{h}") for h in range(NC3)]
        xq_dram = dram.tile([c.T, c.H], F16, tag="xq_dram")
        acc_dram = dram.tile([c.T, c.HL], F32, tag="acc_dram")
        dscr = dram.tile([c.T], F32, tag="dscr")
        cscr = dram.tile([c.T], F32, tag="cscr")

        wsums = const.tile([1, 4], F32, tag="wsums")  # f, i, g, o totals
        d_all = const.tile([128, c.MT], F32, tag="d_all")  # 1/scale_tok cols

        ssq_s_cols = const.tile([128, c.MT], F32, tag="ssq_s_cols")
        ssq_u_cols = const.tile([128, c.MT], F32, tag="ssq_u_cols")
        vmax_cols = const.tile([128, c.MT], F32, tag="vmax_cols")

        wi_idx = {"f": 0, "i": 1, "g": 2, "o": 3}
        wcols = const.tile([128, 4 * 8], F32, tag="wcols")  # abs-sum strips

        def wsum_finish(key):
            """Reduce this weight's 8 abs-sum cols to wsums[0, idx], then
            AllGather with the pair partner."""
            idx = wi_idx[key]
            tot = small.tile([128, 1], F32, tag="wfin_tot", name=f"wt_{key}")
            nc.vector.tensor_reduce(
                tot[:], wcols[:, idx * 8:(idx + 1) * 8], AX.X, ALU.add)
            with tc.tile_pool(name="wf_ps", bufs=1, space="PSUM") as pp:
                ps = pp.tile([1, 1], F32, tag="wf_ps", name=f"wfp_{key}")
                nc.tensor.matmul(ps[:], ones_col[:], tot[:], start=True,
                                 stop=True)
                nc.scalar.copy(wsums[0:1, idx:idx + 1], ps[:])
            nc.sync.dma_start(cc1_in[key][:], wsums[0:1, idx:idx + 1])
            nc.gpsimd.collective_compute(
                "AllGather", ALU.bypass, replica_groups=c.pairs,
                ins=[cc1_in[key].opt()], outs=[cc1_out[key].opt()])

        def bcast_col(src_ap, tag):
            t = const.tile([128, 1], F32, tag=tag)
            with tc.tile_pool(name="bc_ps", bufs=1, space="PSUM") as pp:
                ps = pp.tile([128, 1], F32, tag="bc_ps")
                nc.tensor.matmul(ps[:], ones_row[:], src_ap, start=True,
                                 stop=True)
                nc.scalar.copy(t[:], ps[:])
            return t

        n_w_elems = float(c.H) * float(c.E)

        def finish_scale(key):
            """cc1_out[key] [2,1] -> (s_col, m_col) [128,1] broadcasts."""
            a = small.tile([1, 2], F32, tag="fs_a", name=f"fs_a_{key}")
            nc.sync.dma_start(a[0:1, 0:1], cc1_out[key][0:1, :])
            nc.sync.dma_start(a[0:1, 1:2], cc1_out[key][1:2, :])
            m = const.tile([1, 1], F32, tag=f"fs_m_{key}")
            nc.vector.tensor_reduce(m[:], a[:], AX.X, ALU.add)
            nc.vector.tensor_scalar(m[:], m[:], 1.0 / n_w_elems, 1e-5,
                                    ALU.mult, ALU.max)
            s = const.tile([1, 1], F32, tag=f"fs_s_{key}")
            nc.vector.reciprocal(s[:], m[:])
            return (bcast_col(s[0:1, 0:1], f"sc_{key}"),
                    bcast_col(m[0:1, 0:1], f"mc_{key}"))

        # ===== long-lived big tiles (scoped: closed before P4b/P5) =====
        stats_cm = tc.tile_pool(name="stats", bufs=1)
        stats = stats_cm.__enter__()
        sq_acc_s = stats.tile([128, c.T], F32, tag="sq_acc_s")
        sq_acc_u = stats.tile([128, c.T], F32, tag="sq_acc_u")
        vmax = stats.tile([128, c.T], F32, tag="vmax")

        xqTp_cm = tc.tile_pool(name="xqTp", bufs=1)
        xqTp = xqTp_cm.__enter__()
        xqT_t = xqTp.tile([128, c.KH, c.T], F16, tag="xqT")

        # ------------------------------------------------------------------
        # Preamble: x act_quant (PE-transposed into xqT) interleaved with
        # weight |.| strip sums.  Strip stream order: f(8) i(8) g(8) o(8),
        # 2 strips per m-chunk; scale collectives fire as each weight's
        # strips complete.
        # ------------------------------------------------------------------
        def w_strip_dma(wt, key, s):
            if key == "o":
                # woT [E, HL]: 2 k-rows per strip -> [128, 2*HL]
                for a in range(2):
                    nc.sync.dma_start(
                        wt[:, a * c.HL:(a + 1) * c.HL],
                        woT[s * 256 + a * 128:s * 256 + (a + 1) * 128, :])
            else:
                nc.sync.dma_start(wt[:], wb[key][:, s, :])

        # f/i/g strips first (3 per m-chunk for m<8) so the f scale lands
        # by m=2 and all three by m=7; wo strips ride m=8..15
        strip_stream = [(k, s) for k in ("f", "i", "g", "o") for s in range(8)]

        def strips_for_m(m):
            if m < 8:
                return [si for si in (3 * m, 3 * m + 1, 3 * m + 2)
                        if si < 24]
            return [24 + (m - 8)]

        prio_anchor = [None]
        with tc.tile_pool(name="xp", bufs=5) as xp, \
             tc.tile_pool(name="xsq", bufs=2) as xsqp, \
             tc.tile_pool(name="xq16", bufs=2) as xqp, \
             tc.tile_pool(name="wstr", bufs=2) as wstr, \
             tc.tile_pool(name="pcol", bufs=2) as pcol, \
             tc.tile_pool(name="tp_ps", bufs=4, space="PSUM") as tpp:

            for ng in range(c.MT // 4):
                if ng == 2:
                    # P3's emission is priority-anchored here so the
                    # scheduler can interleave its early j's with the
                    # remaining preamble groups
                    prio_anchor[0] = tc.cur_priority
                xts = []
                ssq4 = pcol.tile([128, 4], F32, tag="ssq4", name="ssq4")
                amax4 = pcol.tile([128, 4], F32, tag="amax4", name="amax4")
                for mi in range(4):
                    m = 4 * ng + mi
                    xt = xp.tile([128, c.H], F32, tag="x_t", name="x_t")
                    nc.sync.dma_start(xt[:], x[m * 128:(m + 1) * 128, :])
                    xts.append(xt)
                    sq = xsqp.tile([128, c.H], F32, tag="x_sq", name="x_sq")
                    nc.scalar.activation(sq[:], xt[:], AF.Square,
                                         accum_out=ssq4[:, mi:mi + 1])
                    nc.vector.tensor_reduce(amax4[:, mi:mi + 1], xt[:], AX.X,
                                            ALU.max, apply_absolute_value=True)
                    # weight abs strips (3/m for m<8, then 1/m)
                    for si in strips_for_m(m):
                        wkey, s = strip_stream[si]
                        wt = wstr.tile([128, 2048], F32, tag="wabs_t",
                                       name="wabs_t")
                        w_strip_dma(wt, wkey, s)
                        # |w| in place on ACT, per-partition sum via accum
                        nc.scalar.activation(
                            wt[:], wt[:], AF.Abs,
                            accum_out=wcols[:, wi_idx[wkey] * 8 + s:
                                            wi_idx[wkey] * 8 + s + 1])
                        if s == 7:
                            wsum_finish(wkey)

                # per-group stat chain on [128, 4]
                v = pcol.tile([128, 4], F32, tag="x_v", name="x_v")
                nc.vector.tensor_scalar(v[:], ssq4[:], 1.0 / c.H, 1e-8,
                                        ALU.mult, ALU.add)
                rv = pcol.tile([128, 4], F32, tag="x_rv", name="x_rv")
                nc.vector.reciprocal(rv[:], v[:])
                r0 = pcol.tile([128, 4], F32, tag="x_r0", name="x_r0")
                nc.scalar.sqrt(r0[:], rv[:])
                nt = pcol.tile([128, 4], F32, tag="x_nt", name="x_nt")
                nc.vector.tensor_tensor(nt[:], r0[:], r0[:], ALU.mult)
                nc.vector.tensor_tensor(nt[:], nt[:], v[:], ALU.mult)
                nc.vector.tensor_scalar(nt[:], nt[:], -0.5, 1.5,
                                        ALU.mult, ALU.add)
                rstd = pcol.tile([128, 4], F32, tag="x_rstd", name="x_rstd")
                nc.vector.tensor_tensor(rstd[:], r0[:], nt[:], ALU.mult)
                amx = pcol.tile([128, 4], F32, tag="x_amx", name="x_amx")
                nc.vector.tensor_tensor(amx[:], amax4[:], rstd[:], ALU.mult)
                nc.vector.tensor_scalar(amx[:], amx[:], 1e-5, None, ALU.max)
                ra = pcol.tile([128, 4], F32, tag="x_ra", name="x_ra")
                nc.vector.reciprocal(ra[:], amx[:])
                sc = pcol.tile([128, 4], F32, tag="x_sc", name="x_sc")
                nc.vector.tensor_scalar(sc[:], ra[:], 127.0, None, ALU.mult)
                cc4 = pcol.tile([128, 4], F32, tag="x_cc", name="x_cc")
                nc.vector.tensor_tensor(cc4[:], sc[:], rstd[:], ALU.mult)
                nc.vector.reciprocal(d_all[:, 4 * ng:4 * ng + 4], sc[:])

                for mi in range(4):
                    m = 4 * ng + mi
                    xt = xts[mi]
                    xqf = xsqp.tile([128, c.H], F32, tag="x_sq", name="xqf")
                    nc.vector.tensor_scalar(xqf[:], xt[:], cc4[:, mi:mi + 1],
                                            M32, ALU.mult, ALU.add)
                    # fold the per-token dequant 1/scale into the quantized
                    # activations: xq = (round(x*cc)) * d_tok, fp16
                    xq = xqp.tile([128, c.H], F16, tag="xq16", name="xq16")
                    nc.vector.tensor_scalar(xq[:], xqf[:], M32, None,
                                            ALU.subtract)
                    nc.vector.tensor_scalar(xq[:], xq[:], d_all[:, m:m + 1],
                                            None, ALU.mult)
                    if "d1_xq" in outs:
                        nc.sync.dma_start(
                            outs["d1_xq"][m * 128:(m + 1) * 128, :], xq[:])
                    # PE transpose into xqT, 4 k-blocks per PSUM bank
                    for kq in range(c.KH // 4):
                        ps = tpp.tile([128, 512], F16, tag="tp_ps",
                                      name="tp")
                        for i4 in range(4):
                            k = 4 * kq + i4
                            nc.tensor.transpose(
                                ps[:, i4 * 128:(i4 + 1) * 128],
                                xq[:, k * 128:(k + 1) * 128], ident16[:])
                        nc.vector.tensor_copy(
                            xqT_t[:, 4 * kq:4 * kq + 4,
                                  m * 128:(m + 1) * 128],
                            ps[:].rearrange("p (a b) -> p a b", b=128))

        s_wf_c, m_wf_c = finish_scale("f")
        s_wi_c, m_wi_c = finish_scale("i")
        s_wg_c, m_wg_c = finish_scale("g")
        s_wo_c, m_wo_c = finish_scale("o")

        # ------------------------------------------------------------------
        # P3: per local-e chunk: W strips, i/f/g matmuls, gates, scan, u,
        #     stat accumulation.  u spilled to DRAM.
        # ------------------------------------------------------------------
        p3_prio = ExitStack()
        if prio_anchor[0] is not None and tc.cur_priority > prio_anchor[0]:
            p3_prio.enter_context(
                tc.high_priority(offset=tc.cur_priority - prio_anchor[0]))
        with p3_prio, \
             tc.tile_pool(name="p3", bufs=2) as p3, \
             tc.tile_pool(name="p3w", bufs=4) as p3w, \
             tc.tile_pool(name="p3q", bufs=4) as p3q, \
             tc.tile_pool(name="p3s", bufs=3) as p3s, \
             tc.tile_pool(name="p3a", bufs=6) as p3a, \
             tc.tile_pool(name="mm_ps", bufs=8, space="PSUM") as mm_ps:

            def quant_w_strip(key, s_col, j, nm):
                strip = p3w.tile([128, c.KH * 128], F32, tag="w_strip",
                                 name="w_strip")
                nc.sync.dma_start(strip[:], wb[key][:, j, :])
                nc.scalar.activation(strip[:], strip[:], AF.Copy, bias=M32,
                                     scale=s_col[:])
                nc.vector.tensor_scalar(strip[:], strip[:], M32, 1.0,
                                        ALU.subtract, ALU.min)
                q = p3q.tile([128, c.KH, 128], F16, tag="wq_strip", name=nm)
                nc.vector.tensor_scalar(q[:].rearrange("p k e -> p (k e)"),
                                        strip[:], -1.0, None, ALU.max)
                return q

            def proj_all(wq):
                # k-outer so each LDWEIGHTS covers NN matmuls
                pss = [mm_ps.tile([128, c.NT], F32, tag="proj_ps",
                                  name=f"proj_ps{n}") for n in range(c.NN)]
                for k in range(c.KH):
                    for n in range(c.NN):
                        nc.tensor.matmul(
                            pss[n][:], wq[:, k, :],
                            xqT_t[:, k, n * c.NT:(n + 1) * c.NT],
                            start=(k == 0), stop=(k == c.KH - 1),
                            skip_group_check=True)
                return pss

            for j in range(c.JE):
                wq_f = quant_w_strip("f", s_wf_c, j, "wq_f")
                wq_i = quant_w_strip("i", s_wi_c, j, "wq_i")
                wq_g = quant_w_strip("g", s_wg_c, j, "wq_g")

                f_j = p3.tile([128, c.T], F32, tag="bigA", name="f_j")
                ii_j = p3.tile([128, c.T], F32, tag="bigB", name="ii_j")
                pss_f = proj_all(wq_f)
                fms = []
                for n in range(c.NN):  # sigmoid batch (direct from PSUM)
                    sl = bass.ts(n, c.NT)
                    nc.scalar.activation(f_j[:, sl], pss_f[n][:], AF.Sigmoid,
                                         scale=m_wf_c[:])
                    fm = p3a.tile([128, c.NT], F32, tag="act_o", name="fm")
                    nc.vector.tensor_scalar(fm[:], f_j[:, sl], -1.0, 1.0,
                                            ALU.mult, ALU.add)
                    fms.append(fm)
                pss_i = proj_all(wq_i)
                for n in range(c.NN):  # silu batch + ii
                    sl = bass.ts(n, c.NT)
                    si = p3a.tile([128, c.NT], F32, tag="act_o", name="si")
                    nc.scalar.activation(si[:], pss_i[n][:], AF.Silu,
                                         scale=m_wi_c[:])
                    nc.vector.tensor_tensor(ii_j[:, sl], si[:], fms[n][:],
                                            ALU.mult)

                s_j = p3.tile([128, c.T], F32, tag="bigB", name="s_j")
                nc.vector.tensor_tensor_scan(s_j[:], f_j[:], ii_j[:],
                                             0.0, ALU.mult, ALU.add)
                if "d2_f" in outs:
                    nc.sync.dma_start(
                        outs["d2_f"][j * 128:(j + 1) * 128, :], f_j[:])
                if "d3_s" in outs:
                    nc.sync.dma_start(
                        outs["d3_s"][j * 128:(j + 1) * 128, :], s_j[:])

                u_j = p3.tile([128, c.T], F32, tag="bigA", name="u_j")
                pss_g = proj_all(wq_g)
                for n in range(c.NN):  # silu batch (g) + u
                    sl = bass.ts(n, c.NT)
                    gg = p3a.tile([128, c.NT], F32, tag="act_o", name="gg")
                    nc.scalar.activation(gg[:], pss_g[n][:], AF.Silu,
                                         scale=m_wg_c[:])
                    nc.vector.tensor_tensor(u_j[:, sl], gg[:], s_j[:, sl],
                                            ALU.mult)
                    if not c.ones_rms:
                        nc.vector.tensor_scalar(u_j[:, sl], u_j[:, sl],
                                                rms_cols[:, j:j + 1],
                                                None, ALU.mult)
                for n in range(c.NN):  # square batch + stat acc
                    sl = bass.ts(n, c.NT)
                    sq1 = p3s.tile([128, c.NT], F32, tag="sq_scr", name="sq1")
                    nc.scalar.activation(sq1[:], s_j[:, sl], AF.Square)
                    if j == 0:
                        nc.vector.tensor_copy(sq_acc_s[:, sl], sq1[:])
                    else:
                        nc.vector.tensor_tensor(sq_acc_s[:, sl],
                                                sq_acc_s[:, sl], sq1[:],
                                                ALU.add)
                    sq2 = p3s.tile([128, c.NT], F32, tag="sq_scr", name="sq2")
                    nc.scalar.activation(sq2[:], u_j[:, sl], AF.Square)
                    if j == 0:
                        nc.vector.tensor_copy(sq_acc_u[:, sl], sq2[:])
                        if c.ones_no:
                            nc.vector.tensor_copy(vmax[:, sl], sq2[:])
                        else:
                            nc.vector.tensor_scalar(
                                vmax[:, sl], sq2[:], no2_cols[:, 0:1],
                                None, ALU.mult)
                    else:
                        nc.vector.tensor_tensor(sq_acc_u[:, sl],
                                                sq_acc_u[:, sl], sq2[:],
                                                ALU.add)
                        if c.ones_no:
                            nc.vector.tensor_tensor(vmax[:, sl], vmax[:, sl],
                                                    sq2[:], ALU.max)
                        else:
                            va2 = p3s.tile([128, c.NT], F32, tag="sq_scr",
                                           name="va2")
                            nc.vector.tensor_scalar(
                                va2[:], sq2[:], no2_cols[:, j:j + 1],
                                None, ALU.mult)
                            nc.vector.tensor_tensor(vmax[:, sl], vmax[:, sl],
                                                    va2[:], ALU.max)
                nc.sync.dma_start(u_dram[j * 128:(j + 1) * 128, :], u_j[:])
                if "d4_u" in outs:
                    nc.sync.dma_start(
                        outs["d4_u"][j * 128:(j + 1) * 128, :], u_j[:])
            p3_end_prio = tc.cur_priority
        tc.cur_priority = max(tc.cur_priority, p3_end_prio)

        xqTp_cm.__exit__(None, None, None)  # free xqT's 8 MB

        # --------------------------------------------------------------
        # P4a: partition-reduce stats -> per-token columns
        # (before woq so the stats pool can close in stack order)
        # --------------------------------------------------------------
        with tc.tile_pool(name="st_ps", bufs=4, space="PSUM") as stp:
            for src_t, dst, op in ((sq_acc_s, ssq_s_cols, ALU.add),
                                   (sq_acc_u, ssq_u_cols, ALU.add),
                                   (vmax, vmax_cols, ALU.max)):
                for mq in range(c.MT // 4):
                    tp = stp.tile([128, 512], F32, tag="st_ps", name="tp")
                    for i4 in range(4):
                        m = 4 * mq + i4
                        nc.tensor.transpose(
                            tp[:, i4 * 128:(i4 + 1) * 128],
                            src_t[:, m * 128:(m + 1) * 128], ident32[:])
                    nc.vector.tensor_reduce(
                        dst[:, 4 * mq:4 * mq + 4],
                        tp[:].rearrange("p (a b) -> p a b", b=128),
                        AX.X, op)
        stats_cm.__exit__(None, None, None)  # free the 4 MB of stat tiles

        # --------------------------------------------------------------
        # woq quant: Wo full half -> ternary fp16 [128, KE, HL]
        # --------------------------------------------------------------
        woqp = ctx.enter_context(tc.tile_pool(name="woqp", bufs=1))
        woq = woqp.tile([128, c.KE, c.HL], F16, tag="woq")
        with tc.tile_pool(name="wo_ld", bufs=2) as wol:
            for k in range(c.KE):
                wt = wol.tile([128, c.HL], F32, tag="wo_t", name="wo_t")
                nc.sync.dma_start(wt[:], woT[k * 128:(k + 1) * 128, :])
                nc.scalar.activation(wt[:], wt[:], AF.Copy, bias=M32,
                                     scale=s_wo_c[:])
                nc.vector.tensor_scalar(wt[:], wt[:], M32, 1.0,
                                        ALU.subtract, ALU.min)
                nc.vector.tensor_scalar(woq[:, k, :], wt[:], -1.0, None,
                                        ALU.max)

        # ssq exchange fires first; the amax sqrt-chain and its (smaller)
        # exchange overlap it, and the rstd chain overlaps that
        for row, cols in ((0, ssq_s_cols), (1, ssq_u_cols)):
            nc.sync.dma_start(
                cc2x_in[row, :].rearrange("(m p) -> p m", p=128), cols[:])
        nc.gpsimd.collective_compute(
            "AllGather", ALU.bypass, replica_groups=c.pairs,
            ins=[cc2x_in.opt()], outs=[cc2x_out.opt()])

        amax_cols = const.tile([128, c.MT], F32, tag="amax_cols")
        a0 = const.tile([128, c.MT], F32, tag="amax_a0")
        nc.scalar.sqrt(a0[:], vmax_cols[:])
        ar = const.tile([128, c.MT], F32, tag="amax_ar")
        nc.vector.tensor_scalar(ar[:], a0[:], 1e-30, None, ALU.max)
        nc.vector.reciprocal(ar[:], ar[:])
        nc.vector.tensor_tensor(ar[:], ar[:], vmax_cols[:], ALU.mult)
        nc.vector.tensor_tensor(ar[:], ar[:], a0[:], ALU.add)
        nc.vector.tensor_scalar(amax_cols[:], ar[:], 0.5, None, ALU.mult)
        nc.sync.dma_start(
            cc2y_in[0, :].rearrange("(m p) -> p m", p=128), amax_cols[:])
        nc.gpsimd.collective_compute(
            "AllGather", ALU.bypass, replica_groups=c.pairs,
            ins=[cc2y_in.opt()], outs=[cc2y_out.opt()])

        def load_stat_cols(cc_out, row, op, tag):
            a = small.tile([128, c.MT], F32, tag=tag + "_a", name=tag + "_a")
            b = small.tile([128, c.MT], F32, tag=tag + "_b", name=tag + "_b")
            nc.sync.dma_start(a[:],
                              cc_out[0, row, :].rearrange("(m p) -> p m",
                                                          p=128))
            nc.sync.dma_start(b[:],
                              cc_out[1, row, :].rearrange("(m p) -> p m",
                                                          p=128))
            r = small.tile([128, c.MT], F32, tag=tag, name=tag)
            nc.vector.tensor_tensor(r[:], a[:], b[:], op)
            return r

        def refine_rsqrt_cols(v_ap, r0_ap, out_ap, tag):
            nt = small.tile([128, c.MT], F32, tag=tag)
            nc.vector.tensor_tensor(nt[:], r0_ap, r0_ap, ALU.mult)
            nc.vector.tensor_tensor(nt[:], nt[:], v_ap, ALU.mult)
            nc.vector.tensor_scalar(nt[:], nt[:], -0.5, 1.5, ALU.mult,
                                    ALU.add)
            nc.vector.tensor_tensor(out_ap, r0_ap, nt[:], ALU.mult)

        ssq_s = load_stat_cols(cc2x_out, 0, ALU.add, "ssq_s")
        ssq_u = load_stat_cols(cc2x_out, 1, ALU.add, "ssq_u")
        amax_y = load_stat_cols(cc2y_out, 0, ALU.max, "amax_y")

        ms = small.tile([128, c.MT], F32, tag="ms")
        nc.vector.tensor_scalar(ms[:], ssq_s[:], 1.0 / c.E, 1e-5, ALU.mult,
                                ALU.add)
        rms_i = small.tile([128, c.MT], F32, tag="rms_i")
        nc.vector.reciprocal(rms_i[:], ms[:])
        rstd_s0 = small.tile([128, c.MT], F32, tag="rstd_s0")
        nc.scalar.sqrt(rstd_s0[:], rms_i[:])
        rstd_s = small.tile([128, c.MT], F32, tag="rstd_s")
        refine_rsqrt_cols(ms[:], rstd_s0[:], rstd_s[:], "nt_s")

        m2 = small.tile([128, c.MT], F32, tag="m2")
        nc.vector.tensor_scalar(m2[:], ssq_u[:], 1.0 / c.E, None, ALU.mult)
        r2 = small.tile([128, c.MT], F32, tag="r2")
        nc.vector.tensor_tensor(r2[:], rstd_s[:], rstd_s[:], ALU.mult)
        nc.vector.tensor_tensor(m2[:], m2[:], r2[:], ALU.mult)
        nc.vector.tensor_scalar(m2[:], m2[:], 1e-8, None, ALU.add)
        m2i = small.tile([128, c.MT], F32, tag="m2i")
        nc.vector.reciprocal(m2i[:], m2[:])
        rsty0 = small.tile([128, c.MT], F32, tag="rsty0")
        nc.scalar.sqrt(rsty0[:], m2i[:])
        rsty = small.tile([128, c.MT], F32, tag="rsty")
        refine_rsqrt_cols(m2[:], rsty0[:], rsty[:], "nt_y")

        rr = small.tile([128, c.MT], F32, tag="rr")
        nc.vector.tensor_tensor(rr[:], rstd_s[:], rsty[:], ALU.mult)
        av = small.tile([128, c.MT], F32, tag="av")
        nc.vector.tensor_tensor(av[:], amax_y[:], rr[:], ALU.mult)
        nc.vector.tensor_scalar(av[:], av[:], 1e-5, None, ALU.max)
        avi = small.tile([128, c.MT], F32, tag="avi")
        nc.vector.reciprocal(avi[:], av[:])
        sc_y = small.tile([128, c.MT], F32, tag="sc_y")
        nc.vector.tensor_scalar(sc_y[:], avi[:], 127.0, None, ALU.mult)
        c_y = small.tile([128, c.MT], F32, tag="c_y")
        nc.vector.tensor_tensor(c_y[:], rr[:], sc_y[:], ALU.mult)
        d_y = const.tile([128, c.MT], F32, tag="d_y")
        nc.vector.reciprocal(d_y[:], sc_y[:])
        nc.vector.tensor_scalar(d_y[:], d_y[:], m_wo_c[:], None, ALU.mult)

        if not c.ones_no:
            nc.sync.dma_start(cscr[:].rearrange("(m p) -> p m", p=128),
                              c_y[:])
        ones128 = const.tile([128, 128], F32, tag="ones128")
        nc.vector.memset(ones128[:], 1.0)

        # ------------------------------------------------------------------
        # P4b + P5: quantize y per strip, chunked AllGather, two-pass final
        # matmul (pass A: local strips from SBUF with wave-0 pipelining;
        # pass B: remote strips = row0 + row1 - local).
        # ------------------------------------------------------------------
        with tc.tile_pool(name="yq", bufs=c.JE) as yqp, \
             tc.tile_pool(name="yq8p", bufs=2) as yq8p, \
             tc.tile_pool(name="rem", bufs=c.JE) as remp, \
             tc.tile_pool(name="accp", bufs=1) as accp, \
             tc.tile_pool(name="u_rdp", bufs=4) as urdp, \
             tc.tile_pool(name="ccrd", bufs=2) as ccrd, \
             tc.tile_pool(name="out_sb", bufs=3) as osb, \
             tc.tile_pool(name="nc_ps", bufs=2, space="PSUM") as ncp, \
             tc.tile_pool(name="out_ps", bufs=4, space="PSUM") as ops:

            c_row = None
            if not c.ones_no:
                c_row = const.tile([1, c.T], F32, tag="c_row")
                nc.sync.dma_start(c_row[0:1, :],
                                  cscr[:].rearrange("(a t) -> a t", a=1))

            ncb = None
            if c.ones_no:
                # norm_o == 1: the quant scale c_y broadcast is
                # j-independent; build it by PE-transposing per-partition
                # broadcasts of the c_y columns (no DRAM bounce)
                ncb = accp.tile([128, c.T], F32, tag="ncb")
                for m in range(c.MT):
                    rb = osb.tile([128, 128], F32, tag="ncb_rb", name="rb")
                    nc.vector.tensor_scalar(rb[:], ones128[:],
                                            c_y[:, m:m + 1], None, ALU.mult)
                    ps_nc = ncp.tile([128, 128], F32, tag="nc_ps",
                                     name="nc_ps")
                    nc.tensor.transpose(ps_nc[:], rb[:], ident32[:])
                    nc.scalar.copy(ncb[:, m * 128:(m + 1) * 128], ps_nc[:])

            yqs = []
            yq8s = []
            rems = []
            for j in range(c.JE):
                yq_j = yqp.tile([128, c.T], F16, tag="yq_j", name=f"yq_{j}")
                yqs.append(yq_j)
                for n in range(c.NN):
                    sl = bass.ts(n, c.NT)
                    u_rd = urdp.tile([128, c.NT], F32, tag="u_rd",
                                     name="u_rd")
                    nc.sync.dma_start(u_rd[:],
                                      u_dram[j * 128:(j + 1) * 128, sl])
                    q0 = osb.tile([128, c.NT], F32, tag="q0", name="q0")
                    if c.ones_no:
                        nc.vector.tensor_tensor(q0[:], u_rd[:], ncb[:, sl],
                                                ALU.mult)
                    else:
                        ps_nc = ncp.tile([128, c.NT], F32, tag="nc_ps",
                                         name="nc_ps")
                        nc.tensor.matmul(
                            ps_nc[:], norm_o_row[0:1, j * 128:(j + 1) * 128],
                            c_row[0:1, sl], start=True, stop=True)
                        nc.vector.tensor_tensor(q0[:], u_rd[:], ps_nc[:],
                                                ALU.mult)
                    nc.vector.tensor_scalar(yq_j[:, sl], q0[:], M32, M32,
                                            ALU.add, ALU.subtract)
                if "d5_yq" in outs:
                    nc.sync.dma_start(
                        outs["d5_yq"][j * 128:(j + 1) * 128, :], yq_j[:])
                # int8 wire format halves the AllGather bytes
                yq8 = yq8p.tile([128, c.T], I8, tag="yq8_j", name="yq8_j")
                nc.vector.tensor_copy(yq8[:], yq_j[:])
                ch = j // J_CH
                jr = j % J_CH
                nc.sync.dma_start(
                    cc3_in[ch][jr * 128:(jr + 1) * 128, :], yq8[:])
                if jr == J_CH - 1:
                    nc.gpsimd.collective_compute(
                        "AllGather", ALU.bypass, replica_groups=c.pairs,
                        ins=[cc3_in[ch].opt()], outs=[cc3_out[ch].opt()])

            # blend remote strips = row0 + row1 - local (exact fp16 ints);
            # emitted after the whole quant loop so the waiting blends don't
            # block the strict-FIFO DVE queue
            for jj in range(c.JE):
                ch, r = jj // J_CH, jj % J_CH
                r0 = ccrd.tile([128, c.T], I8, tag="ccrd", name="r0")
                nc.sync.dma_start(
                    r0[:], cc3_out[ch][0, r * 128:(r + 1) * 128, :])
                r1 = ccrd.tile([128, c.T], I8, tag="ccrd", name="r1")
                nc.sync.dma_start(
                    r1[:], cc3_out[ch][1, r * 128:(r + 1) * 128, :])
                c0 = ccrd.tile([128, c.T], F16, tag="ccup", name="c0")
                nc.vector.tensor_copy(c0[:], r0[:])
                c1 = ccrd.tile([128, c.T], F16, tag="ccup", name="c1")
                nc.vector.tensor_copy(c1[:], r1[:])
                rem = remp.tile([128, c.T], F16, tag="rem_j",
                                name=f"rem_{jj}")
                nc.vector.tensor_tensor(rem[:], c0[:], c1[:], ALU.add)
                nc.vector.tensor_tensor(rem[:], rem[:], yqs[jj][:],
                                        ALU.subtract)
                rems.append(rem)

            # two-pass final matmul: all of pass A (local strips, no wire
            # dependency) before pass B (remote strips via AllGather+blend).
            # n2-inner so each stationary yq/rem chunk serves 2 matmuls.
            NH2 = c.HL // c.NT
            for m in range(c.MT):
                msl = bass.ts(m, 128)
                psA = [ops.tile([128, c.NT], F32, tag="out_ps",
                                name=f"pA{n2}") for n2 in range(NH2)]
                for j in range(c.JE):
                    for n2 in range(NH2):
                        nc.tensor.matmul(
                            psA[n2][:], yqs[j][:, msl],
                            woq[:, j, n2 * c.NT:(n2 + 1) * c.NT],
                            start=(j == 0), stop=(j == c.JE - 1),
                            skip_group_check=True)
                for n2 in range(NH2):
                    nsl = bass.ts(n2, c.NT)
                    aw = osb.tile([128, c.NT], F32, tag="acc_w", name="aw")
                    nc.scalar.copy(aw[:], psA[n2][:])
                    nc.sync.dma_start(acc_dram[msl, nsl], aw[:])
            for m in range(c.MT):
                msl = bass.ts(m, 128)
                psB = [ops.tile([128, c.NT], F32, tag="out_ps",
                                name=f"pB{n2}") for n2 in range(NH2)]
                for j in range(c.JE):
                    for n2 in range(NH2):
                        nc.tensor.matmul(
                            psB[n2][:], rems[j][:, msl],
                            woq[:, c.JE + j, n2 * c.NT:(n2 + 1) * c.NT],
                            start=(j == 0), stop=(j == c.JE - 1),
                            skip_group_check=True)
                for n2 in range(NH2):
                    nsl = bass.ts(n2, c.NT)
                    ar = osb.tile([128, c.NT], F32, tag="acc_r", name="ar")
                    nc.sync.dma_start(ar[:], acc_dram[msl, nsl])
                    ot = osb.tile([128, c.NT], F32, tag="out_t", name="out_t")
                    nc.vector.tensor_tensor(ot[:], psB[n2][:],
                                            ar[:], ALU.add)
                    ot2 = osb.tile([128, c.NT], F32, tag="out_t2",
                                   name="out_t2")
                    nc.scalar.activation(ot2[:], ot[:], AF.Copy,
                                         scale=d_y[:, m:m + 1])
                    nc.sync.dma_start(out[msl, nsl], ot2[:])


# ----------------------------------------------------------------------
# Host wrapper
# ----------------------------------------------------------------------
_CACHE = {}


def _build_full_program(cfg: Cfg):
    nc = bacc.Bacc(None, target_bir_lowering=False, debug=False,
                   num_devices=cfg.n_cores)
    ins_h = {
        "x": nc.dram_tensor("x", [cfg.T, cfg.H], F32, kind="ExternalInput"),
        "wbi": nc.dram_tensor("wbi", [128, cfg.JE, cfg.KH * 128], F32,
                              kind="ExternalInput"),
        "wbf": nc.dram_tensor("wbf", [128, cfg.JE, cfg.KH * 128], F32,
                              kind="ExternalInput"),
        "wbg": nc.dram_tensor("wbg", [128, cfg.JE, cfg.KH * 128], F32,
                              kind="ExternalInput"),
        "woT": nc.dram_tensor("woT", [cfg.E, cfg.HL], F32,
                              kind="ExternalInput"),
        "rms_w_h": nc.dram_tensor("rms_w_h", [cfg.EL], F32,
                                  kind="ExternalInput"),
        "norm_o_h": nc.dram_tensor("norm_o_h", [cfg.EL], F32,
                                   kind="ExternalInput"),
    }
    out_h = nc.dram_tensor("out", [cfg.T, cfg.HL], F32, kind="ExternalOutput")
    outs = {"out": out_h[:, :]}
    import os
    if os.environ.get("HGRN_DEBUG"):
        for nm, shape, dt in (("d1_xq", [cfg.T, cfg.H], F16),
                              ("d2_f", [cfg.EL, cfg.T], F32),
                              ("d3_s", [cfg.EL, cfg.T], F32),
                              ("d4_u", [cfg.EL, cfg.T], F32),
                              ("d5_yq", [cfg.EL, cfg.T], F16)):
            h = nc.dram_tensor(nm, shape, dt, kind="ExternalOutput")
            outs[nm] = h[:, :]
    with tile.TileContext(nc) as tc:
        build_hgrn(tc, outs,
                   {k: v[tuple(slice(None) for _ in v.shape)]
                    for k, v in ins_h.items()}, cfg)
    nc.compile()
    return nc


def _block_w(w_half_T, cfg):
    """[H, EL] -> [128, JE, KH*128] so strip j is 8KB/partition contiguous."""
    kh, je = cfg.KH, cfg.JE
    return np.ascontiguousarray(
        w_half_T.reshape(kh, 128, je, 128).transpose(1, 2, 0, 3)
        .reshape(128, je, kh * 128))


def make_in_maps(x, Wi, Wf, Wg, Wo, rms_w, norm_o, cfg: Cfg):
    in_maps = []
    for core in range(cfg.n_cores):
        b, eh = core // 2, core % 2
        esl = slice(eh * cfg.EL, (eh + 1) * cfg.EL)
        hsl = slice(eh * cfg.HL, (eh + 1) * cfg.HL)
        woT_full = np.ascontiguousarray(Wo[hsl, :].T)  # [E, HL], global order
        loc = woT_full[eh * cfg.EL:(eh + 1) * cfg.EL]
        rmt = woT_full[(1 - eh) * cfg.EL:(2 - eh) * cfg.EL]
        in_maps.append({
            "x": np.ascontiguousarray(x[b]),
            "wbi": _block_w(np.ascontiguousarray(Wi[esl, :].T), cfg),
            "wbf": _block_w(np.ascontiguousarray(Wf[esl, :].T), cfg),
            "wbg": _block_w(np.ascontiguousarray(Wg[esl, :].T), cfg),
            "woT": np.ascontiguousarray(np.concatenate([loc, rmt], axis=0)),
            "rms_w_h": np.ascontiguousarray(rms_w[esl]),
            "norm_o_h": np.ascontiguousarray(norm_o[esl]),
        })
    return in_maps


def kernel(x, Wi, Wf, Wg, Wo, norm_i, norm_f, norm_g, norm_o, rms_w,
           _trace=False):
    x = np.asarray(x, np.float32)
    for nv in (norm_i, norm_f, norm_g):
        if not np.allclose(np.asarray(nv), 1.0):
            raise NotImplementedError(
                "kernel assumes norm_i == norm_f == norm_g == 1 "
                "(as produced by setup_inputs)")
    B, L, H = x.shape
    cfg = Cfg(T=L, H=H, EL=np.asarray(Wi).shape[0] // 2, n_cores=8,
              ones_rms=bool(np.allclose(np.asarray(rms_w), 1.0)),
              ones_no=bool(np.allclose(np.asarray(norm_o), 1.0)))
    assert B * 2 == cfg.n_cores

    from concourse import bass_utils

    key = (cfg.T, cfg.H, cfg.EL, cfg.ones_rms, cfg.ones_no)
    if key not in _CACHE:
        _CACHE[key] = _build_full_program(cfg)
    nc = _CACHE[key]

    in_maps = make_in_maps(np.asarray(x, np.float32),
                           np.asarray(Wi, np.float32),
                           np.asarray(Wf, np.float32),
                           np.asarray(Wg, np.float32),
                           np.asarray(Wo, np.float32),
                           np.asarray(rms_w, np.float32),
                           np.asarray(norm_o, np.float32), cfg)
    res = bass_utils.run_bass_kernel_spmd(
        nc, in_maps, core_ids=list(range(cfg.n_cores)), trace=_trace)

    out = np.empty((B, L, H), np.float32)
    for core in range(cfg.n_cores):
        b, eh = core // 2, core % 2
        out[b, :, eh * cfg.HL:(eh + 1) * cfg.HL] = res.results[core]["out"]
    kernel.last_raw = res.results
    if _trace:
        kernel.last_exec_time_ns = res.exec_time_ns
        kernel.last_results = res
    return out


# revision 86
# speedup vs baseline: 1.0714x; 1.0209x over previous
"""Trainium2 Bass kernel for MinimalHGRNCore (BitLinear projections + HGRN scan).

Contract: kernel(**inputs) takes FULL unsharded numpy inputs and returns the
FULL (B, L, H) float32 output.

Sharding: 8 cores = (batch b in 0..3) x (E-half eh in 0..1).
Each core processes all L tokens of one batch and half of the E features for
the i/f/g projections + recurrence; the final Wo projection is split by
output-H half, contracting over full E.  The remote-half quantized y
activations arrive via a pair-wise AllGather; the final matmul runs in two
passes (local half from SBUF while the collective flies, then the remote
half) so the PE never waits on the wire.

Exactness: act_quant produces integers in [-127,127] and weight_quant values
in {-1,0,+1} * scales.  Both are exactly representable in fp16, so the PE
matmuls run in fp16 with fp32 PSUM accumulation == exact integer arithmetic
(|sum| <= 2048*127 < 2^24).  Rounding uses the fp32 magic-number trick
(x + 1.5*2^23 rounds to nearest-even integer), matching jnp.round.

Layout notes (host side, in make_in_maps):
- Wi/Wf/Wg halves ship as block layout [128, JE, KH*128] so each P3 j-strip
  DMA is 8 KB/partition contiguous.
- Wo half ships as [E, HL] with the core's LOCAL E-half rows first, so the
  two-pass final matmul addresses its weight strips core-independently.
"""

from contextlib import ExitStack
from dataclasses import dataclass

import numpy as np

import concourse.bass as bass
import concourse.mybir as mybir
import concourse.tile as tile
from concourse import bacc
from concourse.masks import make_identity

F32 = mybir.dt.float32
F16 = mybir.dt.float16
AF = mybir.ActivationFunctionType
ALU = mybir.AluOpType
AX = mybir.AxisListType

M32 = 12582912.0  # 1.5 * 2**23: fp32 add rounds to nearest-even integer exactly


@dataclass
class Cfg:
    T: int = 2048      # tokens per core (= L of its batch)
    H: int = 2048      # input hidden dim (contraction for i/f/g)
    EL: int = 1024     # local E features per core (= E/2)
    n_cores: int = 8
    silu_lut: bool = True
    ones_rms: bool = False   # rms_w == 1 -> skip the s*rms multiply
    ones_no: bool = False    # norm_o == 1 -> skip |u|*norm_o^2 scaling

    @property
    def E(self):
        return 2 * self.EL

    @property
    def HL(self):
        return self.H // 2

    @property
    def MT(self):
        return self.T // 128

    @property
    def KH(self):
        return self.H // 128

    @property
    def JE(self):
        return self.EL // 128

    @property
    def KE(self):
        return self.E // 128

    @property
    def NT(self):
        return min(512, self.T)

    @property
    def NN(self):
        return self.T // self.NT

    @property
    def pairs(self):
        return [[2 * i, 2 * i + 1] for i in range(self.n_cores // 2)]


def build_hgrn(tc: tile.TileContext, outs: dict, ins: dict, cfg: Cfg):
    """Emit the SPMD program (identical on every core) into TileContext tc."""
    nc = tc.nc
    c = cfg
    x = ins["x"]
    wb = {"f": ins["wbf"], "i": ins["wbi"], "g": ins["wbg"]}
    woT = ins["woT"]
    rms_w_h, norm_o_h = ins["rms_w_h"], ins["norm_o_h"]
    out = outs["out"]

    ctx = ExitStack()
    with ctx:
        const = ctx.enter_context(tc.tile_pool(name="const", bufs=1))
        small = ctx.enter_context(tc.tile_pool(name="small", bufs=2))
        dram = ctx.enter_context(tc.tile_pool(name="dram", bufs=1, space="DRAM"))

        ones_row = const.tile([1, 128], F32, tag="ones_row")
        nc.vector.memset(ones_row[:], 1.0)
        ones_col = const.tile([128, 1], F32, tag="ones_col")
        nc.vector.memset(ones_col[:], 1.0)
        ident16 = const.tile([128, 128], F16, tag="ident16")
        make_identity(nc, ident16[:])
        ident32 = const.tile([128, 128], F32, tag="ident32")
        make_identity(nc, ident32[:])

        norm_o_row = const.tile([1, c.EL], F32, tag="norm_o_row")
        nc.sync.dma_start(norm_o_row[0:1, :],
                          norm_o_h[:].rearrange("(a t) -> a t", a=1))
        rms_cols = const.tile([128, c.JE], F32, tag="rms_cols")
        norm_o_cols = const.tile([128, c.JE], F32, tag="norm_o_cols")
        nc.sync.dma_start(rms_cols[:],
                          rms_w_h[:].rearrange("(j p) -> p j", p=128))
        nc.sync.dma_start(norm_o_cols[:],
                          norm_o_h[:].rearrange("(j p) -> p j", p=128))
        no2_cols = const.tile([128, c.JE], F32, tag="no2_cols")
        nc.vector.tensor_tensor(no2_cols[:], norm_o_cols[:], norm_o_cols[:],
                                ALU.mult)

        # ------------------------------------------------------------------
        # DRAM bounce tensors for the collectives + spills
        # ------------------------------------------------------------------
        cc1_in = {k: dram.tile([1, 1], F32, tag=f"cc1i_{k}", name=f"cc1i_{k}")
                  for k in ("f", "i", "g", "o")}
        cc1_out = {k: dram.tile([2, 1], F32, tag=f"cc1o_{k}", name=f"cc1o_{k}")
                   for k in ("f", "i", "g", "o")}
        u_dram = dram.tile([c.EL, c.T], F32, tag="u_dram")
        cc2x_in = dram.tile([2, c.T], F32, tag="cc2x_in")
        cc2x_out = dram.tile([2, 2, c.T], F32, tag="cc2x_out")
        cc2y_in = dram.tile([1, c.T], F32, tag="cc2y_in")
        cc2y_out = dram.tile([2, 1, c.T], F32, tag="cc2y_out")
        I8 = mybir.dt.int8
        NC3 = 2  # number of cc3 chunks
        J_CH = c.JE // NC3  # strips per chunk
        cc3_in = [dram.tile([J_CH * 128, c.T], I8, tag=f"cc3i{h}",
                            name=f"cc3i{h}") for h in range(NC3)]
        cc3_out = [dram.tile([2, J_CH * 128, c.T], I8, tag=f"cc3o{h}",
                             name=f"cc3o{h}") for h in range(NC3)]
        xq_dram = dram.tile([c.T, c.H], F16, tag="xq_dram")
        acc_dram = dram.tile([c.T, c.HL], F32, tag="acc_dram")
        dscr = dram.tile([c.T], F32, tag="dscr")
        cscr = dram.tile([c.T], F32, tag="cscr")

        wsums = const.tile([1, 4], F32, tag="wsums")  # f, i, g, o totals
        d_all = const.tile([128, c.MT], F32, tag="d_all")  # 1/scale_tok cols

        ssq_s_cols = const.tile([128, c.MT], F32, tag="ssq_s_cols")
        ssq_u_cols = const.tile([128, c.MT], F32, tag="ssq_u_cols")
        vmax_cols = const.tile([128, c.MT], F32, tag="vmax_cols")

        wi_idx = {"f": 0, "i": 1, "g": 2, "o": 3}
        wcols = const.tile([128, 4 * 8], F32, tag="wcols")  # abs-sum strips

        def wsum_finish(key):
            """Reduce this weight's 8 abs-sum cols to wsums[0, idx], then
            AllGather with the pair partner."""
            idx = wi_idx[key]
            tot = small.tile([128, 1], F32, tag="wfin_tot", name=f"wt_{key}")
            nc.vector.tensor_reduce(
                tot[:], wcols[:, idx * 8:(idx + 1) * 8], AX.X, ALU.add)
            with tc.tile_pool(name="wf_ps", bufs=1, space="PSUM") as pp:
                ps = pp.tile([1, 1], F32, tag="wf_ps", name=f"wfp_{key}")
                nc.tensor.matmul(ps[:], ones_col[:], tot[:], start=True,
                                 stop=True)
                nc.scalar.copy(wsums[0:1, idx:idx + 1], ps[:])
            nc.sync.dma_start(cc1_in[key][:], wsums[0:1, idx:idx + 1])
            nc.gpsimd.collective_compute(
                "AllGather", ALU.bypass, replica_groups=c.pairs,
                ins=[cc1_in[key].opt()], outs=[cc1_out[key].opt()])

        def bcast_col(src_ap, tag):
            t = const.tile([128, 1], F32, tag=tag)
            with tc.tile_pool(name="bc_ps", bufs=1, space="PSUM") as pp:
                ps = pp.tile([128, 1], F32, tag="bc_ps")
                nc.tensor.matmul(ps[:], ones_row[:], src_ap, start=True,
                                 stop=True)
                nc.scalar.copy(t[:], ps[:])
            return t

        n_w_elems = float(c.H) * float(c.E)

        def finish_scale(key):
            """cc1_out[key] [2,1] -> (s_col, m_col) [128,1] broadcasts."""
            a = small.tile([1, 2], F32, tag="fs_a", name=f"fs_a_{key}")
            nc.sync.dma_start(a[0:1, 0:1], cc1_out[key][0:1, :])
            nc.sync.dma_start(a[0:1, 1:2], cc1_out[key][1:2, :])
            m = const.tile([1, 1], F32, tag=f"fs_m_{key}")
            nc.vector.tensor_reduce(m[:], a[:], AX.X, ALU.add)
            nc.vector.tensor_scalar(m[:], m[:], 1.0 / n_w_elems, 1e-5,
                                    ALU.mult, ALU.max)
            s = const.tile([1, 1], F32, tag=f"fs_s_{key}")
            nc.vector.reciprocal(s[:], m[:])
            return (bcast_col(s[0:1, 0:1], f"sc_{key}"),
                    bcast_col(m[0:1, 0:1], f"mc_{key}"))

        # ===== long-lived big tiles (scoped: closed before P4b/P5) =====
        stats_cm = tc.tile_pool(name="stats", bufs=1)
        stats = stats_cm.__enter__()
        sq_acc_s = stats.tile([128, c.T], F32, tag="sq_acc_s")
        sq_acc_u = stats.tile([128, c.T], F32, tag="sq_acc_u")
        vmax = stats.tile([128, c.T], F32, tag="vmax")

        xqTp_cm = tc.tile_pool(name="xqTp", bufs=1)
        xqTp = xqTp_cm.__enter__()
        xqT_t = xqTp.tile([128, c.KH, c.T], F16, tag="xqT")

        # ------------------------------------------------------------------
        # Preamble: x act_quant (PE-transposed into xqT) interleaved with
        # weight |.| strip sums.  Strip stream order: f(8) i(8) g(8) o(8),
        # 2 strips per m-chunk; scale collectives fire as each weight's
        # strips complete.
        # ------------------------------------------------------------------
        def w_strip_dma(wt, key, s):
            if key == "o":
                # woT [E, HL]: 2 k-rows per strip -> [128, 2*HL]
                for a in range(2):
                    nc.sync.dma_start(
                        wt[:, a * c.HL:(a + 1) * c.HL],
                        woT[s * 256 + a * 128:s * 256 + (a + 1) * 128, :])
            else:
                nc.sync.dma_start(wt[:], wb[key][:, s, :])

        # f/i/g strips first (3 per m-chunk for m<8) so the f scale lands
        # by m=2 and all three by m=7; wo strips ride m=8..15
        strip_stream = [(k, s) for k in ("f", "i", "g", "o") for s in range(8)]

        def strips_for_m(m):
            if m < 8:
                return [si for si in (3 * m, 3 * m + 1, 3 * m + 2)
                        if si < 24]
            return [24 + (m - 8)]

        prio_anchor = [None]
        with tc.tile_pool(name="xp", bufs=5) as xp, \
             tc.tile_pool(name="xsq", bufs=2) as xsqp, \
             tc.tile_pool(name="xq16", bufs=2) as xqp, \
             tc.tile_pool(name="wstr", bufs=2) as wstr, \
             tc.tile_pool(name="pcol", bufs=2) as pcol, \
             tc.tile_pool(name="tp_ps", bufs=4, space="PSUM") as tpp:

            for ng in range(c.MT // 4):
                if ng == 2:
                    # P3's emission is priority-anchored here so the
                    # scheduler can interleave its early j's with the
                    # remaining preamble groups
                    prio_anchor[0] = tc.cur_priority
                xts = []
                ssq4 = pcol.tile([128, 4], F32, tag="ssq4", name="ssq4")
                amax4 = pcol.tile([128, 4], F32, tag="amax4", name="amax4")
                for mi in range(4):
                    m = 4 * ng + mi
                    xt = xp.tile([128, c.H], F32, tag="x_t", name="x_t")
                    nc.sync.dma_start(xt[:], x[m * 128:(m + 1) * 128, :])
                    xts.append(xt)
                    sq = xsqp.tile([128, c.H], F32, tag="x_sq", name="x_sq")
                    nc.scalar.activation(sq[:], xt[:], AF.Square,
                                         accum_out=ssq4[:, mi:mi + 1])
                    nc.vector.tensor_reduce(amax4[:, mi:mi + 1], xt[:], AX.X,
                                            ALU.max, apply_absolute_value=True)
                    # weight abs strips (3/m for m<8, then 1/m)
                    for si in strips_for_m(m):
                        wkey, s = strip_stream[si]
                        wt = wstr.tile([128, 2048], F32, tag="wabs_t",
                                       name="wabs_t")
                        w_strip_dma(wt, wkey, s)
                        # |w| in place on ACT, per-partition sum via accum
                        nc.scalar.activation(
                            wt[:], wt[:], AF.Abs,
                            accum_out=wcols[:, wi_idx[wkey] * 8 + s:
                                            wi_idx[wkey] * 8 + s + 1])
                        if s == 7:
                            wsum_finish(wkey)

                # per-group stat chain on [128, 4]
                v = pcol.tile([128, 4], F32, tag="x_v", name="x_v")
                nc.vector.tensor_scalar(v[:], ssq4[:], 1.0 / c.H, 1e-8,
                                        ALU.mult, ALU.add)
                rv = pcol.tile([128, 4], F32, tag="x_rv", name="x_rv")
                nc.vector.reciprocal(rv[:], v[:])
                r0 = pcol.tile([128, 4], F32, tag="x_r0", name="x_r0")
                nc.scalar.sqrt(r0[:], rv[:])
                nt = pcol.tile([128, 4], F32, tag="x_nt", name="x_nt")
                nc.vector.tensor_tensor(nt[:], r0[:], r0[:], ALU.mult)
                nc.vector.tensor_tensor(nt[:], nt[:], v[:], ALU.mult)
                nc.vector.tensor_scalar(nt[:], nt[:], -0.5, 1.5,
                                        ALU.mult, ALU.add)
                rstd = pcol.tile([128, 4], F32, tag="x_rstd", name="x_rstd")
                nc.vector.tensor_tensor(rstd[:], r0[:], nt[:], ALU.mult)
                amx = pcol.tile([128, 4], F32, tag="x_amx", name="x_amx")
                nc.vector.tensor_tensor(amx[:], amax4[:], rstd[:], ALU.mult)
                nc.vector.tensor_scalar(amx[:], amx[:], 1e-5, None, ALU.max)
                ra = pcol.tile([128, 4], F32, tag="x_ra", name="x_ra")
                nc.vector.reciprocal(ra[:], amx[:])
                sc = pcol.tile([128, 4], F32, tag="x_sc", name="x_sc")
                nc.vector.tensor_scalar(sc[:], ra[:], 127.0, None, ALU.mult)
                cc4 = pcol.tile([128, 4], F32, tag="x_cc", name="x_cc")
                nc.vector.tensor_tensor(cc4[:], sc[:], rstd[:], ALU.mult)
                nc.vector.reciprocal(d_all[:, 4 * ng:4 * ng + 4], sc[:])

                for mi in range(4):
                    m = 4 * ng + mi
                    xt = xts[mi]
                    xqf = xsqp.tile([128, c.H], F32, tag="x_sq", name="xqf")
                    nc.vector.tensor_scalar(xqf[:], xt[:], cc4[:, mi:mi + 1],
                                            M32, ALU.mult, ALU.add)
                    # fold the per-token dequant 1/scale into the quantized
                    # activations: xq = (round(x*cc)) * d_tok, fp16
                    xq = xqp.tile([128, c.H], F16, tag="xq16", name="xq16")
                    nc.vector.tensor_scalar(xq[:], xqf[:], M32, None,
                                            ALU.subtract)
                    nc.vector.tensor_scalar(xq[:], xq[:], d_all[:, m:m + 1],
                                            None, ALU.mult)
                    if "d1_xq" in outs:
                        nc.sync.dma_start(
                            outs["d1_xq"][m * 128:(m + 1) * 128, :], xq[:])
                    # PE transpose into xqT, 4 k-blocks per PSUM bank
                    for kq in range(c.KH // 4):
                        ps = tpp.tile([128, 512], F16, tag="tp_ps",
                                      name="tp")
                        for i4 in range(4):
                            k = 4 * kq + i4
                            nc.tensor.transpose(
                                ps[:, i4 * 128:(i4 + 1) * 128],
                                xq[:, k * 128:(k + 1) * 128], ident16[:])
                        nc.vector.tensor_copy(
                            xqT_t[:, 4 * kq:4 * kq + 4,
                                  m * 128:(m + 1) * 128],
                            ps[:].rearrange("p (a b) -> p a b", b=128))

        s_wf_c, m_wf_c = finish_scale("f")
        s_wi_c, m_wi_c = finish_scale("i")
        s_wg_c, m_wg_c = finish_scale("g")
        s_wo_c, m_wo_c = finish_scale("o")

        # ------------------------------------------------------------------
        # P3: per local-e chunk: W strips, i/f/g matmuls, gates, scan, u,
        #     stat accumulation.  u spilled to DRAM.
        # ------------------------------------------------------------------
        p3_prio = ExitStack()
        if prio_anchor[0] is not None and tc.cur_priority > prio_anchor[0]:
            p3_prio.enter_context(
                tc.high_priority(offset=tc.cur_priority - prio_anchor[0]))
        with p3_prio, \
             tc.tile_pool(name="p3", bufs=2) as p3, \
             tc.tile_pool(name="p3w", bufs=2) as p3w, \
             tc.tile_pool(name="p3q", bufs=4) as p3q, \
             tc.tile_pool(name="p3s", bufs=3) as p3s, \
             tc.tile_pool(name="p3a", bufs=6) as p3a, \
             tc.tile_pool(name="mm_ps", bufs=8, space="PSUM") as mm_ps:

            def quant_w_strip(key, s_col, j, nm):
                strip = p3w.tile([128, c.KH * 128], F32, tag="w_strip",
                                 name="w_strip")
                nc.sync.dma_start(strip[:], wb[key][:, j, :])
                nc.scalar.activation(strip[:], strip[:], AF.Copy, bias=M32,
                                     scale=s_col[:])
                nc.vector.tensor_scalar(strip[:], strip[:], M32, 1.0,
                                        ALU.subtract, ALU.min)
                q = p3q.tile([128, c.KH, 128], F16, tag="wq_strip", name=nm)
                nc.vector.tensor_scalar(q[:].rearrange("p k e -> p (k e)"),
                                        strip[:], -1.0, None, ALU.max)
                return q

            def proj_all(wq):
                # k-outer so each LDWEIGHTS covers NN matmuls
                pss = [mm_ps.tile([128, c.NT], F32, tag="proj_ps",
                                  name=f"proj_ps{n}") for n in range(c.NN)]
                for k in range(c.KH):
                    for n in range(c.NN):
                        nc.tensor.matmul(
                            pss[n][:], wq[:, k, :],
                            xqT_t[:, k, n * c.NT:(n + 1) * c.NT],
                            start=(k == 0), stop=(k == c.KH - 1),
                            skip_group_check=True)
                return pss

            for j in range(c.JE):
                wq_f = quant_w_strip("f", s_wf_c, j, "wq_f")
                wq_i = quant_w_strip("i", s_wi_c, j, "wq_i")
                wq_g = quant_w_strip("g", s_wg_c, j, "wq_g")

                f_j = p3.tile([128, c.T], F32, tag="bigA", name="f_j")
                ii_j = p3.tile([128, c.T], F32, tag="bigB", name="ii_j")
                pss_f = proj_all(wq_f)
                fms = []
                for n in range(c.NN):  # sigmoid batch (direct from PSUM)
                    sl = bass.ts(n, c.NT)
                    nc.scalar.activation(f_j[:, sl], pss_f[n][:], AF.Sigmoid,
                                         scale=m_wf_c[:])
                    fm = p3a.tile([128, c.NT], F32, tag="act_o", name="fm")
                    nc.vector.tensor_scalar(fm[:], f_j[:, sl], -1.0, 1.0,
                                            ALU.mult, ALU.add)
                    fms.append(fm)
                pss_i = proj_all(wq_i)
                for n in range(c.NN):  # silu batch + ii
                    sl = bass.ts(n, c.NT)
                    si = p3a.tile([128, c.NT], F32, tag="act_o", name="si")
                    nc.scalar.activation(si[:], pss_i[n][:], AF.Silu,
                                         scale=m_wi_c[:])
                    nc.vector.tensor_tensor(ii_j[:, sl], si[:], fms[n][:],
                                            ALU.mult)

                s_j = p3.tile([128, c.T], F32, tag="bigB", name="s_j")
                nc.vector.tensor_tensor_scan(s_j[:], f_j[:], ii_j[:],
                                             0.0, ALU.mult, ALU.add)
                if "d2_f" in outs:
                    nc.sync.dma_start(
                        outs["d2_f"][j * 128:(j + 1) * 128, :], f_j[:])
                if "d3_s" in outs:
                    nc.sync.dma_start(
                        outs["d3_s"][j * 128:(j + 1) * 128, :], s_j[:])

                u_j = p3.tile([128, c.T], F32, tag="bigA", name="u_j")
                pss_g = proj_all(wq_g)
                for n in range(c.NN):  # silu batch (g) + u
                    sl = bass.ts(n, c.NT)
                    gg = p3a.tile([128, c.NT], F32, tag="act_o", name="gg")
                    nc.scalar.activation(gg[:], pss_g[n][:], AF.Silu,
                                         scale=m_wg_c[:])
                    nc.vector.tensor_tensor(u_j[:, sl], gg[:], s_j[:, sl],
                                            ALU.mult)
                    if not c.ones_rms:
                        nc.vector.tensor_scalar(u_j[:, sl], u_j[:, sl],
                                                rms_cols[:, j:j + 1],
                                                None, ALU.mult)
                for n in range(c.NN):  # square batch + stat acc
                    sl = bass.ts(n, c.NT)
                    sq1 = p3s.tile([128, c.NT], F32, tag="sq_scr", name="sq1")
                    nc.scalar.activation(sq1[:], s_j[:, sl], AF.Square)
                    if j == 0:
                        nc.vector.tensor_copy(sq_acc_s[:, sl], sq1[:])
                    else:
                        nc.vector.tensor_tensor(sq_acc_s[:, sl],
                                                sq_acc_s[:, sl], sq1[:],
                                                ALU.add)
                    sq2 = p3s.tile([128, c.NT], F32, tag="sq_scr", name="sq2")
                    nc.scalar.activation(sq2[:], u_j[:, sl], AF.Square)
                    if j == 0:
                        nc.vector.tensor_copy(sq_acc_u[:, sl], sq2[:])
                        if c.ones_no:
                            nc.vector.tensor_copy(vmax[:, sl], sq2[:])
                        else:
                            nc.vector.tensor_scalar(
                                vmax[:, sl], sq2[:], no2_cols[:, 0:1],
                                None, ALU.mult)
                    else:
                        nc.vector.tensor_tensor(sq_acc_u[:, sl],
                                                sq_acc_u[:, sl], sq2[:],
                                                ALU.add)
                        if c.ones_no:
                            nc.vector.tensor_tensor(vmax[:, sl], vmax[:, sl],
                                                    sq2[:], ALU.max)
                        else:
                            va2 = p3s.tile([128, c.NT], F32, tag="sq_scr",
                                           name="va2")
                            nc.vector.tensor_scalar(
                                va2[:], sq2[:], no2_cols[:, j:j + 1],
                                None, ALU.mult)
                            nc.vector.tensor_tensor(vmax[:, sl], vmax[:, sl],
                                                    va2[:], ALU.max)
                nc.sync.dma_start(u_dram[j * 128:(j + 1) * 128, :], u_j[:])
                if "d4_u" in outs:
                    nc.sync.dma_start(
                        outs["d4_u"][j * 128:(j + 1) * 128, :], u_j[:])
            p3_end_prio = tc.cur_priority
        tc.cur_priority = max(tc.cur_priority, p3_end_prio)

        xqTp_cm.__exit__(None, None, None)  # free xqT's 8 MB

        # --------------------------------------------------------------
        # P4a: partition-reduce stats -> per-token columns
        # (before woq so the stats pool can close in stack order)
        # --------------------------------------------------------------
        with tc.tile_pool(name="st_ps", bufs=4, space="PSUM") as stp:
            for src_t, dst, op in ((sq_acc_s, ssq_s_cols, ALU.add),
                                   (sq_acc_u, ssq_u_cols, ALU.add),
                                   (vmax, vmax_cols, ALU.max)):
                for mq in range(c.MT // 4):
                    tp = stp.tile([128, 512], F32, tag="st_ps", name="tp")
                    for i4 in range(4):
                        m = 4 * mq + i4
                        nc.tensor.transpose(
                            tp[:, i4 * 128:(i4 + 1) * 128],
                            src_t[:, m * 128:(m + 1) * 128], ident32[:])
                    nc.vector.tensor_reduce(
                        dst[:, 4 * mq:4 * mq + 4],
                        tp[:].rearrange("p (a b) -> p a b", b=128),
                        AX.X, op)
        stats_cm.__exit__(None, None, None)  # free the 4 MB of stat tiles

        # --------------------------------------------------------------
        # woq quant: Wo full half -> ternary fp16 [128, KE, HL]
        # --------------------------------------------------------------
        woqp = ctx.enter_context(tc.tile_pool(name="woqp", bufs=1))
        woq = woqp.tile([128, c.KE, c.HL], F16, tag="woq")
        with tc.tile_pool(name="wo_ld", bufs=2) as wol:
            for k in range(c.KE):
                wt = wol.tile([128, c.HL], F32, tag="wo_t", name="wo_t")
                nc.sync.dma_start(wt[:], woT[k * 128:(k + 1) * 128, :])
                nc.scalar.activation(wt[:], wt[:], AF.Copy, bias=M32,
                                     scale=s_wo_c[:])
                nc.vector.tensor_scalar(wt[:], wt[:], M32, 1.0,
                                        ALU.subtract, ALU.min)
                nc.vector.tensor_scalar(woq[:, k, :], wt[:], -1.0, None,
                                        ALU.max)

        # ssq exchange fires first; the amax sqrt-chain and its (smaller)
        # exchange overlap it, and the rstd chain overlaps that
        for row, cols in ((0, ssq_s_cols), (1, ssq_u_cols)):
            nc.sync.dma_start(
                cc2x_in[row, :].rearrange("(m p) -> p m", p=128), cols[:])
        nc.gpsimd.collective_compute(
            "AllGather", ALU.bypass, replica_groups=c.pairs,
            ins=[cc2x_in.opt()], outs=[cc2x_out.opt()])

        amax_cols = const.tile([128, c.MT], F32, tag="amax_cols")
        a0 = const.tile([128, c.MT], F32, tag="amax_a0")
        nc.scalar.sqrt(a0[:], vmax_cols[:])
        ar = const.tile([128, c.MT], F32, tag="amax_ar")
        nc.vector.tensor_scalar(ar[:], a0[:], 1e-30, None, ALU.max)
        nc.vector.reciprocal(ar[:], ar[:])
        nc.vector.tensor_tensor(ar[:], ar[:], vmax_cols[:], ALU.mult)
        nc.vector.tensor_tensor(ar[:], ar[:], a0[:], ALU.add)
        nc.vector.tensor_scalar(amax_cols[:], ar[:], 0.5, None, ALU.mult)
        nc.sync.dma_start(
            cc2y_in[0, :].rearrange("(m p) -> p m", p=128), amax_cols[:])
        nc.gpsimd.collective_compute(
            "AllGather", ALU.bypass, replica_groups=c.pairs,
            ins=[cc2y_in.opt()], outs=[cc2y_out.opt()])

        def load_stat_cols(cc_out, row, op, tag):
            a = small.tile([128, c.MT], F32, tag=tag + "_a", name=tag + "_a")
            b = small.tile([128, c.MT], F32, tag=tag + "_b", name=tag + "_b")
            nc.sync.dma_start(a[:],
                              cc_out[0, row, :].rearrange("(m p) -> p m",
                                                          p=128))
            nc.sync.dma_start(b[:],
                              cc_out[1, row, :].rearrange("(m p) -> p m",
                                                          p=128))
            r = small.tile([128, c.MT], F32, tag=tag, name=tag)
            nc.vector.tensor_tensor(r[:], a[:], b[:], op)
            return r

        def refine_rsqrt_cols(v_ap, r0_ap, out_ap, tag):
            nt = small.tile([128, c.MT], F32, tag=tag)
            nc.vector.tensor_tensor(nt[:], r0_ap, r0_ap, ALU.mult)
            nc.vector.tensor_tensor(nt[:], nt[:], v_ap, ALU.mult)
            nc.vector.tensor_scalar(nt[:], nt[:], -0.5, 1.5, ALU.mult,
                                    ALU.add)
            nc.vector.tensor_tensor(out_ap, r0_ap, nt[:], ALU.mult)

        ssq_s = load_stat_cols(cc2x_out, 0, ALU.add, "ssq_s")
        ssq_u = load_stat_cols(cc2x_out, 1, ALU.add, "ssq_u")
        amax_y = load_stat_cols(cc2y_out, 0, ALU.max, "amax_y")

        ms = small.tile([128, c.MT], F32, tag="ms")
        nc.vector.tensor_scalar(ms[:], ssq_s[:], 1.0 / c.E, 1e-5, ALU.mult,
                                ALU.add)
        rms_i = small.tile([128, c.MT], F32, tag="rms_i")
        nc.vector.reciprocal(rms_i[:], ms[:])
        rstd_s0 = small.tile([128, c.MT], F32, tag="rstd_s0")
        nc.scalar.sqrt(rstd_s0[:], rms_i[:])
        rstd_s = small.tile([128, c.MT], F32, tag="rstd_s")
        refine_rsqrt_cols(ms[:], rstd_s0[:], rstd_s[:], "nt_s")

        m2 = small.tile([128, c.MT], F32, tag="m2")
        nc.vector.tensor_scalar(m2[:], ssq_u[:], 1.0 / c.E, None, ALU.mult)
        r2 = small.tile([128, c.MT], F32, tag="r2")
        nc.vector.tensor_tensor(r2[:], rstd_s[:], rstd_s[:], ALU.mult)
        nc.vector.tensor_tensor(m2[:], m2[:], r2[:], ALU.mult)
        nc.vector.tensor_scalar(m2[:], m2[:], 1e-8, None, ALU.add)
        m2i = small.tile([128, c.MT], F32, tag="m2i")
        nc.vector.reciprocal(m2i[:], m2[:])
        rsty0 = small.tile([128, c.MT], F32, tag="rsty0")
        nc.scalar.sqrt(rsty0[:], m2i[:])
        rsty = small.tile([128, c.MT], F32, tag="rsty")
        refine_rsqrt_cols(m2[:], rsty0[:], rsty[:], "nt_y")

        rr = small.tile([128, c.MT], F32, tag="rr")
        nc.vector.tensor_tensor(rr[:], rstd_s[:], rsty[:], ALU.mult)
        av = small.tile([128, c.MT], F32, tag="av")
        nc.vector.tensor_tensor(av[:], amax_y[:], rr[:], ALU.mult)
        nc.vector.tensor_scalar(av[:], av[:], 1e-5, None, ALU.max)
        avi = small.tile([128, c.MT], F32, tag="avi")
        nc.vector.reciprocal(avi[:], av[:])
        sc_y = small.tile([128, c.MT], F32, tag="sc_y")
        nc.vector.tensor_scalar(sc_y[:], avi[:], 127.0, None, ALU.mult)
        c_y = small.tile([128, c.MT], F32, tag="c_y")
        nc.vector.tensor_tensor(c_y[:], rr[:], sc_y[:], ALU.mult)
        d_y = const.tile([128, c.MT], F32, tag="d_y")
        nc.vector.reciprocal(d_y[:], sc_y[:])
        nc.vector.tensor_scalar(d_y[:], d_y[:], m_wo_c[:], None, ALU.mult)

        if not c.ones_no:
            nc.sync.dma_start(cscr[:].rearrange("(m p) -> p m", p=128),
                              c_y[:])
        ones128 = const.tile([128, 128], F32, tag="ones128")
        nc.vector.memset(ones128[:], 1.0)

        # ------------------------------------------------------------------
        # P4b + P5: quantize y per strip, chunked AllGather, two-pass final
        # matmul (pass A: local strips from SBUF with wave-0 pipelining;
        # pass B: remote strips = row0 + row1 - local).
        # ------------------------------------------------------------------
        with tc.tile_pool(name="yq", bufs=c.JE) as yqp, \
             tc.tile_pool(name="yq8p", bufs=2) as yq8p, \
             tc.tile_pool(name="rem", bufs=c.JE) as remp, \
             tc.tile_pool(name="accp", bufs=1) as accp, \
             tc.tile_pool(name="u_rdp", bufs=4) as urdp, \
             tc.tile_pool(name="ccrd", bufs=2) as ccrd, \
             tc.tile_pool(name="out_sb", bufs=3) as osb, \
             tc.tile_pool(name="nc_ps", bufs=2, space="PSUM") as ncp, \
             tc.tile_pool(name="out_ps", bufs=4, space="PSUM") as ops:

            c_row = None
            if not c.ones_no:
                c_row = const.tile([1, c.T], F32, tag="c_row")
                nc.sync.dma_start(c_row[0:1, :],
                                  cscr[:].rearrange("(a t) -> a t", a=1))

            ncb = None
            if c.ones_no:
                # norm_o == 1: the quant scale c_y broadcast is
                # j-independent; build it by PE-transposing per-partition
                # broadcasts of the c_y columns (no DRAM bounce)
                ncb = accp.tile([128, c.T], F32, tag="ncb")
                for m in range(c.MT):
                    rb = osb.tile([128, 128], F32, tag="ncb_rb", name="rb")
                    nc.vector.tensor_scalar(rb[:], ones128[:],
                                            c_y[:, m:m + 1], None, ALU.mult)
                    ps_nc = ncp.tile([128, 128], F32, tag="nc_ps",
                                     name="nc_ps")
                    nc.tensor.transpose(ps_nc[:], rb[:], ident32[:])
                    nc.scalar.copy(ncb[:, m * 128:(m + 1) * 128], ps_nc[:])

            yqs = []
            yq8s = []
            rems = []
            for j in range(c.JE):
                yq_j = yqp.tile([128, c.T], F16, tag="yq_j", name=f"yq_{j}")
                yqs.append(yq_j)
                for n in range(c.NN):
                    sl = bass.ts(n, c.NT)
                    u_rd = urdp.tile([128, c.NT], F32, tag="u_rd",
                                     name="u_rd")
                    nc.sync.dma_start(u_rd[:],
                                      u_dram[j * 128:(j + 1) * 128, sl])
                    q0 = osb.tile([128, c.NT], F32, tag="q0", name="q0")
                    if c.ones_no:
                        nc.vector.tensor_tensor(q0[:], u_rd[:], ncb[:, sl],
                                                ALU.mult)
                    else:
                        ps_nc = ncp.tile([128, c.NT], F32, tag="nc_ps",
                                         name="nc_ps")
                        nc.tensor.matmul(
                            ps_nc[:], norm_o_row[0:1, j * 128:(j + 1) * 128],
                            c_row[0:1, sl], start=True, stop=True)
                        nc.vector.tensor_tensor(q0[:], u_rd[:], ps_nc[:],
                                                ALU.mult)
                    nc.vector.tensor_scalar(yq_j[:, sl], q0[:], M32, M32,
                                            ALU.add, ALU.subtract)
                if "d5_yq" in outs:
                    nc.sync.dma_start(
                        outs["d5_yq"][j * 128:(j + 1) * 128, :], yq_j[:])
                # int8 wire format halves the AllGather bytes
                yq8 = yq8p.tile([128, c.T], I8, tag="yq8_j", name="yq8_j")
                nc.vector.tensor_copy(yq8[:], yq_j[:])
                ch = j // J_CH
                jr = j % J_CH
                nc.sync.dma_start(
                    cc3_in[ch][jr * 128:(jr + 1) * 128, :], yq8[:])
                if jr == J_CH - 1:
                    nc.gpsimd.collective_compute(
                        "AllGather", ALU.bypass, replica_groups=c.pairs,
                        ins=[cc3_in[ch].opt()], outs=[cc3_out[ch].opt()])

            # blend remote strips = row0 + row1 - local (exact fp16 ints);
            # emitted after the whole quant loop so the waiting blends don't
            # block the strict-FIFO DVE queue
            for jj in range(c.JE):
                ch, r = jj // J_CH, jj % J_CH
                r0 = ccrd.tile([128, c.T], I8, tag="ccrd", name="r0")
                nc.sync.dma_start(
                    r0[:], cc3_out[ch][0, r * 128:(r + 1) * 128, :])
                r1 = ccrd.tile([128, c.T], I8, tag="ccrd", name="r1")
                nc.sync.dma_start(
                    r1[:], cc3_out[ch][1, r * 128:(r + 1) * 128, :])
                c0 = ccrd.tile([128, c.T], F16, tag="ccup", name="c0")
                nc.vector.tensor_copy(c0[:], r0[:])
                c1 = ccrd.tile([128, c.T], F16, tag="ccup", name="c1")
                nc.vector.tensor_copy(c1[:], r1[:])
                rem = remp.tile([128, c.T], F16, tag="rem_j",
                                name=f"rem_{jj}")
                nc.vector.tensor_tensor(rem[:], c0[:], c1[:], ALU.add)
                nc.vector.tensor_tensor(rem[:], rem[:], yqs[jj][:],
                                        ALU.subtract)
                rems.append(rem)

            # two-pass final matmul: all of pass A (local strips, no wire
            # dependency) before pass B (remote strips via AllGather+blend).
            # n2-inner so each stationary yq/rem chunk serves 2 matmuls.
            NH2 = c.HL // c.NT
            for m in range(c.MT):
                msl = bass.ts(m, 128)
                psA = [ops.tile([128, c.NT], F32, tag="out_ps",
                                name=f"pA{n2}") for n2 in range(NH2)]
                for j in range(c.JE):
                    for n2 in range(NH2):
                        nc.tensor.matmul(
                            psA[n2][:], yqs[j][:, msl],
                            woq[:, j, n2 * c.NT:(n2 + 1) * c.NT],
                            start=(j == 0), stop=(j == c.JE - 1),
                            skip_group_check=True)
                for n2 in range(NH2):
                    nsl = bass.ts(n2, c.NT)
                    aw = osb.tile([128, c.NT], F32, tag="acc_w", name="aw")
                    nc.scalar.copy(aw[:], psA[n2][:])
                    nc.sync.dma_start(acc_dram[msl, nsl], aw[:])
            for m in range(c.MT):
                msl = bass.ts(m, 128)
                psB = [ops.tile([128, c.NT], F32, tag="out_ps",
                                name=f"pB{n2}") for n2 in range(NH2)]
                for j in range(c.JE):
                    for n2 in range(NH2):
                        nc.tensor.matmul(
                            psB[n2][:], rems[j][:, msl],
                            woq[:, c.JE + j, n2 * c.NT:(n2 + 1) * c.NT],
                            start=(j == 0), stop=(j == c.JE - 1),
                            skip_group_check=True)
                for n2 in range(NH2):
                    nsl = bass.ts(n2, c.NT)
                    ar = osb.tile([128, c.NT], F32, tag="acc_r", name="ar")
                    nc.sync.dma_start(ar[:], acc_dram[msl, nsl])
                    ot = osb.tile([128, c.NT], F32, tag="out_t", name="out_t")
                    nc.vector.tensor_tensor(ot[:], psB[n2][:],
                                            ar[:], ALU.add)
                    ot2 = osb.tile([128, c.NT], F32, tag="out_t2",
                                   name="out_t2")
                    nc.scalar.activation(ot2[:], ot[:], AF.Copy,
                                         scale=d_y[:, m:m + 1])
                    nc.sync.dma_start(out[msl, nsl], ot2[:])


# ----------------------------------------------------------------------
# Host wrapper
# ----------------------------------------------------------------------
_CACHE = {}


def _build_full_program(cfg: Cfg):
    nc = bacc.Bacc(None, target_bir_lowering=False, debug=False,
                   num_devices=cfg.n_cores)
    ins_h = {
        "x": nc.dram_tensor("x", [cfg.T, cfg.H], F32, kind="ExternalInput"),
        "wbi": nc.dram_tensor("wbi", [128, cfg.JE, cfg.KH * 128], F32,
                              kind="ExternalInput"),
        "wbf": nc.dram_tensor("wbf", [128, cfg.JE, cfg.KH * 128], F32,
                              kind="ExternalInput"),
        "wbg": nc.dram_tensor("wbg", [128, cfg.JE, cfg.KH * 128], F32,
                              kind="ExternalInput"),
        "woT": nc.dram_tensor("woT", [cfg.E, cfg.HL], F32,
                              kind="ExternalInput"),
        "rms_w_h": nc.dram_tensor("rms_w_h", [cfg.EL], F32,
                                  kind="ExternalInput"),
        "norm_o_h": nc.dram_tensor("norm_o_h", [cfg.EL], F32,
                                   kind="ExternalInput"),
    }
    out_h = nc.dram_tensor("out", [cfg.T, cfg.HL], F32, kind="ExternalOutput")
    outs = {"out": out_h[:, :]}
    import os
    if os.environ.get("HGRN_DEBUG"):
        for nm, shape, dt in (("d1_xq", [cfg.T, cfg.H], F16),
                              ("d2_f", [cfg.EL, cfg.T], F32),
                              ("d3_s", [cfg.EL, cfg.T], F32),
                              ("d4_u", [cfg.EL, cfg.T], F32),
                              ("d5_yq", [cfg.EL, cfg.T], F16)):
            h = nc.dram_tensor(nm, shape, dt, kind="ExternalOutput")
            outs[nm] = h[:, :]
    with tile.TileContext(nc) as tc:
        build_hgrn(tc, outs,
                   {k: v[tuple(slice(None) for _ in v.shape)]
                    for k, v in ins_h.items()}, cfg)
    nc.compile()
    return nc


def _block_w(w_half_T, cfg):
    """[H, EL] -> [128, JE, KH*128] so strip j is 8KB/partition contiguous."""
    kh, je = cfg.KH, cfg.JE
    return np.ascontiguousarray(
        w_half_T.reshape(kh, 128, je, 128).transpose(1, 2, 0, 3)
        .reshape(128, je, kh * 128))


def make_in_maps(x, Wi, Wf, Wg, Wo, rms_w, norm_o, cfg: Cfg):
    in_maps = []
    for core in range(cfg.n_cores):
        b, eh = core // 2, core % 2
        esl = slice(eh * cfg.EL, (eh + 1) * cfg.EL)
        hsl = slice(eh * cfg.HL, (eh + 1) * cfg.HL)
        woT_full = np.ascontiguousarray(Wo[hsl, :].T)  # [E, HL], global order
        loc = woT_full[eh * cfg.EL:(eh + 1) * cfg.EL]
        rmt = woT_full[(1 - eh) * cfg.EL:(2 - eh) * cfg.EL]
        in_maps.append({
            "x": np.ascontiguousarray(x[b]),
            "wbi": _block_w(np.ascontiguousarray(Wi[esl, :].T), cfg),
            "wbf": _block_w(np.ascontiguousarray(Wf[esl, :].T), cfg),
            "wbg": _block_w(np.ascontiguousarray(Wg[esl, :].T), cfg),
            "woT": np.ascontiguousarray(np.concatenate([loc, rmt], axis=0)),
            "rms_w_h": np.ascontiguousarray(rms_w[esl]),
            "norm_o_h": np.ascontiguousarray(norm_o[esl]),
        })
    return in_maps


def kernel(x, Wi, Wf, Wg, Wo, norm_i, norm_f, norm_g, norm_o, rms_w,
           _trace=False):
    x = np.asarray(x, np.float32)
    for nv in (norm_i, norm_f, norm_g):
        if not np.allclose(np.asarray(nv), 1.0):
            raise NotImplementedError(
                "kernel assumes norm_i == norm_f == norm_g == 1 "
                "(as produced by setup_inputs)")
    B, L, H = x.shape
    cfg = Cfg(T=L, H=H, EL=np.asarray(Wi).shape[0] // 2, n_cores=8,
              ones_rms=bool(np.allclose(np.asarray(rms_w), 1.0)),
              ones_no=bool(np.allclose(np.asarray(norm_o), 1.0)))
    assert B * 2 == cfg.n_cores

    from concourse import bass_utils

    key = (cfg.T, cfg.H, cfg.EL, cfg.ones_rms, cfg.ones_no)
    if key not in _CACHE:
        _CACHE[key] = _build_full_program(cfg)
    nc = _CACHE[key]

    in_maps = make_in_maps(np.asarray(x, np.float32),
                           np.asarray(Wi, np.float32),
                           np.asarray(Wf, np.float32),
                           np.asarray(Wg, np.float32),
                           np.asarray(Wo, np.float32),
                           np.asarray(rms_w, np.float32),
                           np.asarray(norm_o, np.float32), cfg)
    res = bass_utils.run_bass_kernel_spmd(
        nc, in_maps, core_ids=list(range(cfg.n_cores)), trace=_trace)

    out = np.empty((B, L, H), np.float32)
    for core in range(cfg.n_cores):
        b, eh = core // 2, core % 2
        out[b, :, eh * cfg.HL:(eh + 1) * cfg.HL] = res.results[core]["out"]
    kernel.last_raw = res.results
    if _trace:
        kernel.last_exec_time_ns = res.exec_time_ns
        kernel.last_results = res
    return out
